# revision 1
# baseline (speedup 1.0000x reference)
"""AdaFace loss on 8 TRN2 NeuronCores — class-parallel margin softmax.

Sharding: class dim split 12500/core. Each core gets its weight shard
pre-transposed ([D, C_loc] f32) for matmul streaming plus the row-major
shard for the label gather. The N x C_loc logits never leave PSUM: each
[125, 512] tile is exp()'d on ACT (with per-class 30/||w|| folded into the
per-partition activation scale) and summed. One small AllReduce combines
per-sample sumexp partials and label logits across cores.
"""
import math
import os
import numpy as np

NCORES = 8
C, D, N = 100000, 512, 512
CLOC = C // NCORES            # 12500
CTILE = 125                   # classes per matmul tile (M)
TPC = 5                       # tiles per weight-load chunk
CCHUNK = CTILE * TPC          # 625
NCHUNK = CLOC // CCHUNK       # 20
NSPLIT = 5                    # pipeline splits
CHUNKS_PER_SPLIT = NCHUNK // NSPLIT
TILES_PER_SPLIT = CHUNKS_PER_SPLIT * TPC
NTILE = N // 128              # 4 batch tiles
SCALEC = 30.0
MARGIN = 0.4
HCONST = 0.333
EXP30 = float(np.exp(np.float32(30.0)))

_cache = {}


def _build():
    import concourse.bass as bass
    import concourse.bacc as bacc
    import concourse.mybir as mybir
    import concourse.tile as tile
    from contextlib import ExitStack

    f32 = mybir.dt.float32
    bf16 = mybir.dt.bfloat16
    fp8 = mybir.dt.float8e4
    i32 = mybir.dt.int32
    AF = mybir.ActivationFunctionType
    OP = mybir.AluOpType

    nc = bacc.Bacc("TRN2", target_bir_lowering=False, debug=False,
                   num_devices=NCORES)
    _c30 = nc.alloc_sbuf_tensor("const-f32-neg30", [128, 1], f32)
    nc.gpsimd.memset(_c30.ap(), -30.0)
    nc.const_aps.aps[(f32, -30.0)] = _c30.ap()
    nc.all_engine_barrier()

    wt_d = nc.dram_tensor("wt", [D, CLOC], f32, kind="ExternalInput")
    wrows_d = nc.dram_tensor("wrows", [CLOC, D], f32, kind="ExternalInput")
    emb_d = nc.dram_tensor("emb", [N, D], f32, kind="ExternalInput")
    labidx_d = nc.dram_tensor("labidx", [128, NTILE], i32, kind="ExternalInput")
    valid_d = nc.dram_tensor("valid", [128, NTILE], f32, kind="ExternalInput")
    identbf_d = nc.dram_tensor("identbf", [128, 128], bf16, kind="ExternalInput")
    identf_d = nc.dram_tensor("identf", [128, 128], f32, kind="ExternalInput")
    onesf_d = nc.dram_tensor("onesf", [128, 1], f32, kind="ExternalInput")
    onesbf_d = nc.dram_tensor("onesbf", [128, 1], bf16, kind="ExternalInput")
    onesrow_d = nc.dram_tensor("onesrow", [1, 128], f32, kind="ExternalInput")
    out_d = nc.dram_tensor("out", [1, 1], f32, kind="ExternalOutput")

    with tile.TileContext(nc) as tc, ExitStack() as ctx:
        constp = ctx.enter_context(tc.tile_pool(name="const", bufs=1))
        wldp = ctx.enter_context(tc.tile_pool(name="wload", bufs=2))
        expp = ctx.enter_context(tc.tile_pool(name="expout", bufs=3))
        smallp = ctx.enter_context(tc.tile_pool(name="small", bufs=2))
        scrp = ctx.enter_context(tc.tile_pool(name="scratch", bufs=2))
        pmain = ctx.enter_context(tc.tile_pool(name="pmain", bufs=3, space="PSUM"))
        pgram = ctx.enter_context(tc.tile_pool(name="pgram", bufs=2, space="PSUM"))
        pmisc = ctx.enter_context(tc.tile_pool(name="pmisc", bufs=2, space="PSUM"))
        dramp = ctx.enter_context(tc.tile_pool(name="dram", bufs=1, space="DRAM"))

        # ---- constants ----
        ident_bf = constp.tile([128, 128], bf16, tag="identbf")
        nc.sync.dma_start(out=ident_bf[:], in_=identbf_d[:, :])
        ident_f = constp.tile([128, 128], f32, tag="identf")
        nc.sync.dma_start(out=ident_f[:], in_=identf_d[:, :])
        ones_f = constp.tile([128, 1], f32, tag="onesf")
        nc.sync.dma_start(out=ones_f[:], in_=onesf_d[:, :])
        ones_bf = constp.tile([128, 1], bf16, tag="onesbf")
        nc.sync.dma_start(out=ones_bf[:], in_=onesbf_d[:, :])
        ones_row = constp.tile([1, 128], f32, tag="onesrow")
        nc.sync.dma_start(out=ones_row[:], in_=onesrow_d[:, :])
        idx_sb = constp.tile([128, NTILE], i32, tag="idx")
        nc.sync.dma_start(out=idx_sb[:], in_=labidx_d[:, :])
        valid_sb = constp.tile([128, NTILE], f32, tag="valid")
        nc.sync.dma_start(out=valid_sb[:], in_=valid_d[:, :])

        def rsqrt(x_ap, y_ap, t_ap, iters=2):
            """y = 1/sqrt(x) via bitcast seed + Newton (x > 0)."""
            xi = x_ap.bitcast(i32)
            yi = y_ap.bitcast(i32)
            nc.vector.tensor_scalar(out=yi, in0=xi, scalar1=1, scalar2=None,
                                    op0=OP.arith_shift_right)
            nc.vector.tensor_scalar(out=yi, in0=yi, scalar1=-1,
                                    scalar2=0x5f3759df, op0=OP.mult, op1=OP.add)
            for _ in range(iters):
                nc.vector.tensor_tensor(out=t_ap, in0=x_ap, in1=y_ap, op=OP.mult)
                nc.vector.tensor_tensor(out=t_ap, in0=t_ap, in1=y_ap, op=OP.mult)
                nc.vector.tensor_scalar(out=t_ap, in0=t_ap, scalar1=-0.5,
                                        scalar2=1.5, op0=OP.mult, op1=OP.add)
                nc.vector.tensor_tensor(out=y_ap, in0=y_ap, in1=t_ap, op=OP.mult)

        # ---- embedding prep ----
        emb_t = []
        for j in range(NTILE):
            t = constp.tile([128, D], f32, tag=f"emb{j}")
            nc.sync.dma_start(out=t[:], in_=emb_d[j * 128:(j + 1) * 128, :])
            emb_t.append(t)

        norms2_b = constp.tile([128, NTILE], f32, tag="norms2")
        for j in range(NTILE):
            scr = scrp.tile([128, D], f32, tag="sq")
            nc.vector.scalar_tensor_tensor(
                out=scr[:], in0=emb_t[j][:], scalar=1.0, in1=emb_t[j][:],
                op0=OP.mult, op1=OP.mult, accum_out=norms2_b[:, j:j + 1])
        invn_b = constp.tile([128, NTILE], f32, tag="invn")
        tmp_b = scrp.tile([128, NTILE], f32, tag="tmpb")
        rsqrt(norms2_b[:], invn_b[:], tmp_b[:])                # 1/||e||
        norms_b = constp.tile([128, NTILE], f32, tag="norms")
        nc.vector.tensor_tensor(out=norms_b[:], in0=norms2_b[:], in1=invn_b[:],
                                op=OP.mult)                    # ||e||

        # normalize embeddings, cast bf16, transpose -> embT[k] = [d128, n512]
        emb_bf = []
        for j in range(NTILE):
            nc.vector.tensor_scalar_mul(emb_t[j][:], emb_t[j][:],
                                        invn_b[:, j:j + 1])
            tb = constp.tile([128, D], bf16, tag=f"embbf{j}")
            nc.vector.tensor_copy(out=tb[:], in_=emb_t[j][:])
            emb_bf.append(tb)
        embT = []
        for k in range(D // 128):
            tT = constp.tile([128, N], bf16, tag=f"embT{k}")
            for j in range(NTILE):
                pst = pmisc.tile([128, 128], bf16, tag="misc")
                nc.tensor.transpose(out=pst[:],
                                    in_=emb_bf[j][:, k * 128:(k + 1) * 128],
                                    identity=ident_bf[:])
                nc.vector.tensor_copy(out=tT[:, j * 128:(j + 1) * 128], in_=pst[:])
            embT.append(tT)

        # ---- main class loop ----
        acc = constp.tile([CTILE, N], bf16, tag="acc")
        nc.vector.memset(acc[:], 0.0)
        acc4 = constp.tile([CTILE, N], bf16, tag="acc4")
        nc.vector.memset(acc4[:], 0.0)
        scales_all = []
        for h in range(NSPLIT):
            sc = constp.tile([CTILE, TILES_PER_SPLIT], f32, tag=f"sc_{h}")
            scales_all.append(sc)
        PLANE = CHUNKS_PER_SPLIT * CCHUNK          # 2500
        PLANEP = ((PLANE + 15) // 16) * 16          # pad so Ko step %16==0
        wf8 = [[constp.tile([128, 2 * PLANEP], fp8,
                            name=f"wf8_{h}_{kk}", tag=f"wf8_{h}_{kk}")
                for kk in range(2)]
               for h in range(NSPLIT)]
        # emb transposed, fp8, x16, planes contiguous: [:, o*512:(o+1)*512]
        embT8 = []
        for kk in range(2):
            t8 = constp.tile([128, 2 * N], fp8, name=f"embT8_{kk}",
                             tag=f"embT8_{kk}")
            for o in range(2):
                nc.vector.tensor_scalar(out=t8[:, o * N:(o + 1) * N],
                                        in0=embT[2 * kk + o][:],
                                        scalar1=16.0, scalar2=None, op0=OP.mult)
            embT8.append(t8)

        _cps = 1 if os.environ.get("K_TINY_MAIN") else CHUNKS_PER_SPLIT
        n2fm = constp.tile([1, CTILE * TILES_PER_SPLIT], f32, tag="n2fm")
        # ---- all weight-load DMAs upfront: one 2.56MB cast-DMA per (h, kk)
        glen = _cps * CCHUNK
        for h in range(NSPLIT):
            g0 = h * CHUNKS_PER_SPLIT * CCHUNK
            for kk in range(2):
                dst = wf8[h][kk][:, :].rearrange(
                    "p (o c) -> p o c", o=2)[:, :, 0:glen]
                src = wt_d[kk * 256:(kk + 1) * 256, g0:g0 + glen].rearrange(
                    "(o p) c -> p o c", o=2)
                nc.gpsimd.dma_start(out=dst, in_=src)

        ones_f8 = constp.tile([128, 32], fp8, tag="onesf8")
        nc.vector.memset(ones_f8[:], 1.0)
        def emit_split(h, acc_t=None):
            acc_t = acc if acc_t is None else acc_t
            # phase 1: squares (ACT kk=0 / DVE kk=1) + fp8 DR gram
            plane = _cps * CCHUNK
            w2 = []
            for kk in range(2):
                w2k = wldp.tile([128, 2 * PLANEP], fp8,
                                name=f"w2_{h}_{kk}", tag=f"w2_{kk}")
                dv = w2k[:, :].rearrange("p (o c) -> p o c", o=2)[:, :, 0:plane]
                sv = wf8[h][kk][:, :].rearrange(
                    "p (o c) -> p o c", o=2)[:, :, 0:plane]
                if kk == 0:
                    nc.scalar.activation(dv, sv, AF.Square)
                else:
                    nc.vector.scalar_tensor_tensor(
                        out=dv, in0=sv, scalar=1.0,
                        in1=sv, op0=OP.mult, op1=OP.mult)
                w2.append(w2k)
            NSUB = plane // 500
            for ns in range(NSUB):
                psn = pgram.tile([1, 500], f32, tag="psn")
                for kk in range(2):
                    rv = w2[kk][:, :].rearrange(
                        "p (o c) -> p o c", o=2)[:, :, ns * 500:(ns + 1) * 500]
                    lv = ones_f8[:, :].rearrange(
                        "p (o c) -> p o c", o=2)[:, :, 0:1]
                    nc.tensor.matmul(out=psn[:], lhsT=lv, rhs=rv,
                                     perf_mode=mybir.MatmulPerfMode.DoubleRow,
                                     start=(kk == 0), stop=(kk == 1))
                fmv = n2fm[0:1, :].rearrange(
                    "a (p t) -> a p t", t=TILES_PER_SPLIT)
                nc.vector.tensor_copy(
                    out=fmv[:, :, ns * 4:ns * 4 + 4],
                    in_=psn[0:1, :].rearrange("a (i p) -> a p i", i=4))
            # phase 2: scales = 30/sqrt(n2) (p-major via reshape DMA)
            n2pm = scrp.tile([CTILE, TILES_PER_SPLIT], f32, tag="n2pm")
            nc.sync.dma_start(out=n2pm[:], in_=n2fm[0:1, :])
            rny = scrp.tile([CTILE, TILES_PER_SPLIT], f32, tag="rny")
            rsqrt(n2pm[:], rny[:], scales_all[h][:])
            nc.vector.tensor_scalar_mul(scales_all[h][:], rny[:],
                                        SCALEC / 16.0)
            # phase 3: matmul + exp + accumulate
            for ti in range(_cps * TPC):
                cs = ti * CTILE
                psm = pmain.tile([CTILE, N], f32, tag="psm")
                for kk in range(2):
                    lv = wf8[h][kk][:, :].rearrange(
                        "p (o c) -> p o c", o=2)[:, :, cs:cs + CTILE]
                    rv = embT8[kk][:, :].rearrange("p (o n) -> p o n", o=2)
                    nc.tensor.matmul(out=psm[:], lhsT=lv, rhs=rv,
                                     perf_mode=mybir.MatmulPerfMode.DoubleRow,
                                     start=(kk == 0), stop=(kk == 1))
                ex = expp.tile([CTILE, N], bf16, tag="ex")
                nc.scalar.activation(ex[:], psm[:], AF.Exp, bias=-30.0,
                                     scale=scales_all[h][:, ti:ti + 1])
                nc.vector.tensor_tensor(out=acc_t[:], in0=acc_t[:], in1=ex[:],
                                        op=OP.add)

        emit_split(0)
        # batch stats: column sums of [norms | norms2] via ones-matmul
        stat_in = smallp.tile([128, 2 * NTILE], f32, tag="statin")
        nc.vector.tensor_copy(out=stat_in[:, 0:NTILE], in_=norms_b[:])
        nc.vector.tensor_copy(out=stat_in[:, NTILE:2 * NTILE], in_=norms2_b[:])
        ps_stat = pmisc.tile([1, 2 * NTILE], f32, tag="misc")
        nc.tensor.matmul(out=ps_stat[:], lhsT=ones_f[:], rhs=stat_in[:],
                         start=True, stop=True)
        sums = smallp.tile([1, 2], f32, tag="sums")
        nc.vector.reduce_sum(out=sums[:, 0:1], in_=ps_stat[:, 0:NTILE],
                             axis=mybir.AxisListType.X)
        nc.vector.reduce_sum(out=sums[:, 1:2], in_=ps_stat[:, NTILE:2 * NTILE],
                             axis=mybir.AxisListType.X)
        # mean = S1/N ; var = (S2 - S1^2/N)/(N-1); std = sqrt(var)
        scal = smallp.tile([1, 4], f32, tag="scal")
        nc.scalar.mul(scal[:, 0:1], sums[:, 0:1], 1.0 / N)            # mean
        nc.vector.tensor_tensor(out=scal[:, 1:2], in0=sums[:, 0:1],
                                in1=sums[:, 0:1], op=OP.mult)         # S1^2
        nc.scalar.mul(scal[:, 1:2], scal[:, 1:2], 1.0 / N)            # S1^2/N
        nc.vector.tensor_tensor(out=scal[:, 1:2], in0=sums[:, 1:2],
                                in1=scal[:, 1:2], op=OP.subtract)
        nc.scalar.mul(scal[:, 1:2], scal[:, 1:2], 1.0 / (N - 1))      # var
        sct = smallp.tile([1, 2], f32, tag="sct")
        rsqrt(scal[:, 1:2], scal[:, 2:3], sct[:, 0:1])                # 1/std
        nc.vector.tensor_tensor(out=scal[:, 2:3], in0=scal[:, 1:2],
                                in1=scal[:, 2:3], op=OP.mult)         # std
        nc.vector.tensor_scalar_add(scal[:, 2:3], scal[:, 2:3], HCONST)  # std+H
        nc.vector.reciprocal(out=scal[:, 3:4], in_=scal[:, 2:3])      # 1/(std+H)
        # broadcast mean, 1/(std+H) to 128 partitions via K=1 outer product
        bvals = smallp.tile([1, 2], f32, tag="bvals")
        nc.vector.tensor_copy(out=bvals[:, 0:1], in_=scal[:, 0:1])
        nc.vector.tensor_copy(out=bvals[:, 1:2], in_=scal[:, 3:4])
        ps_bc = pmisc.tile([128, 2], f32, tag="misc")
        nc.tensor.matmul(out=ps_bc[:], lhsT=ones_row[:], rhs=bvals[:],
                         start=True, stop=True)
        bc = smallp.tile([128, 2], f32, tag="bc")
        nc.vector.tensor_copy(out=bc[:], in_=ps_bc[:])

        # margin scaler -> m, sin(m), cos(m) (poly, avoids Sin table set)
        ms_b = smallp.tile([128, NTILE], f32, tag="msb")
        nc.vector.tensor_tensor(out=ms_b[:], in0=norms_b[:],
                                in1=bc[:, 0:1].to_broadcast([128, NTILE]),
                                op=OP.subtract)
        nc.vector.tensor_tensor(out=ms_b[:], in0=ms_b[:],
                                in1=bc[:, 1:2].to_broadcast([128, NTILE]),
                                op=OP.mult)
        nc.vector.tensor_scalar_min(ms_b[:], ms_b[:], 1.0)
        nc.vector.tensor_scalar_max(ms_b[:], ms_b[:], -1.0)
        m_b = smallp.tile([128, NTILE], f32, tag="mb")
        nc.vector.tensor_scalar(out=m_b[:], in0=ms_b[:], scalar1=MARGIN,
                                scalar2=MARGIN, op0=OP.mult, op1=OP.add)
        u_b = smallp.tile([128, NTILE], f32, tag="ub")
        nc.vector.tensor_tensor(out=u_b[:], in0=m_b[:], in1=m_b[:], op=OP.mult)
        sin_b = smallp.tile([128, NTILE], f32, tag="sinb")
        nc.vector.tensor_scalar(out=sin_b[:], in0=u_b[:], scalar1=1.0 / 120,
                                scalar2=-1.0 / 6, op0=OP.mult, op1=OP.add)
        nc.vector.tensor_tensor(out=sin_b[:], in0=sin_b[:], in1=u_b[:], op=OP.mult)
        nc.vector.tensor_scalar_add(sin_b[:], sin_b[:], 1.0)
        nc.vector.tensor_tensor(out=sin_b[:], in0=sin_b[:], in1=m_b[:], op=OP.mult)
        cos_b = smallp.tile([128, NTILE], f32, tag="cosb")
        nc.vector.tensor_scalar(out=cos_b[:], in0=u_b[:], scalar1=-1.0 / 720,
                                scalar2=1.0 / 24, op0=OP.mult, op1=OP.add)
        nc.vector.tensor_tensor(out=cos_b[:], in0=cos_b[:], in1=u_b[:], op=OP.mult)
        nc.vector.tensor_scalar_add(cos_b[:], cos_b[:], -0.5)
        nc.vector.tensor_tensor(out=cos_b[:], in0=cos_b[:], in1=u_b[:], op=OP.mult)
        nc.vector.tensor_scalar_add(cos_b[:], cos_b[:], 1.0)

        emit_split(1)
        emit_split(2)
        emit_split(3, acc4)
        # ---- label path (part 1: everything except Exp) ----
        wlab_t = []
        for j in range(NTILE):
            wl = constp.tile([128, D], f32, tag=f"wlab{j}")
            if os.environ.get("K_SKIP_GATHER"):
                nc.sync.dma_start(out=wl[:], in_=wrows_d[0:128, :])
            else:
                nc.gpsimd.indirect_dma_start(
                    out=wl[:], out_offset=None, in_=wrows_d[:, :],
                    in_offset=bass.IndirectOffsetOnAxis(ap=idx_sb[:, j:j + 1],
                                                        axis=0))
            wlab_t.append(wl)
        wln2_b = smallp.tile([128, NTILE], f32, tag="wln2")
        dots_b = smallp.tile([128, NTILE], f32, tag="dots")
        for j in range(NTILE):
            scr = scrp.tile([128, D], f32, tag="sq")
            nc.vector.scalar_tensor_tensor(
                out=scr[:], in0=wlab_t[j][:], scalar=1.0, in1=wlab_t[j][:],
                op0=OP.mult, op1=OP.mult, accum_out=wln2_b[:, j:j + 1])
            scr2 = scrp.tile([128, D], f32, tag="sq")
            nc.vector.scalar_tensor_tensor(
                out=scr2[:], in0=emb_t[j][:], scalar=1.0, in1=wlab_t[j][:],
                op0=OP.mult, op1=OP.mult, accum_out=dots_b[:, j:j + 1])
        invwl_b = smallp.tile([128, NTILE], f32, tag="invwl")
        tmp2_b = scrp.tile([128, NTILE], f32, tag="tmpb")
        rsqrt(wln2_b[:], invwl_b[:], tmp2_b[:])
        cost_b = smallp.tile([128, NTILE], f32, tag="cost")
        nc.vector.tensor_tensor(out=cost_b[:], in0=dots_b[:], in1=invwl_b[:],
                                op=OP.mult)
        nc.vector.tensor_scalar_min(cost_b[:], cost_b[:], 1.0)
        nc.vector.tensor_scalar_max(cost_b[:], cost_b[:], -1.0)
        q_b = smallp.tile([128, NTILE], f32, tag="qb")
        nc.vector.tensor_tensor(out=q_b[:], in0=cost_b[:], in1=cost_b[:],
                                op=OP.mult)
        # rt = sqrt(1 - cos^2) = q*rsqrt(q) with q clamped away from 0
        nc.vector.tensor_scalar(out=q_b[:], in0=q_b[:], scalar1=-1.0,
                                scalar2=1.0, op0=OP.mult, op1=OP.add)
        nc.vector.tensor_scalar_max(q_b[:], q_b[:], 1e-12)
        rt_b = smallp.tile([128, NTILE], f32, tag="rtb")
        tmp3_b = scrp.tile([128, NTILE], f32, tag="tmpb")
        rsqrt(q_b[:], rt_b[:], tmp3_b[:])
        nc.vector.tensor_tensor(out=rt_b[:], in0=rt_b[:], in1=q_b[:],
                                op=OP.mult)
        costm_b = smallp.tile([128, NTILE], f32, tag="costm")
        nc.vector.tensor_tensor(out=costm_b[:], in0=cost_b[:], in1=cos_b[:],
                                op=OP.mult)
        nc.vector.tensor_tensor(out=rt_b[:], in0=rt_b[:], in1=sin_b[:],
                                op=OP.mult)
        nc.vector.tensor_tensor(out=costm_b[:], in0=costm_b[:], in1=rt_b[:],
                                op=OP.subtract)
        # ---- label path (part 2: Exp terms) — before the last split ----
        et_b = smallp.tile([128, NTILE], f32, tag="etb")
        nc.scalar.activation(et_b[:], cost_b[:], AF.Exp, bias=-30.0, scale=SCALEC)
        em_b = smallp.tile([128, NTILE], f32, tag="emb2")
        nc.scalar.activation(em_b[:], costm_b[:], AF.Exp, bias=-30.0, scale=SCALEC)
        corr_b = smallp.tile([128, NTILE], f32, tag="corrb")
        nc.vector.tensor_tensor(out=corr_b[:], in0=em_b[:], in1=et_b[:],
                                op=OP.subtract)
        nc.vector.tensor_tensor(out=corr_b[:], in0=corr_b[:], in1=valid_sb[:],
                                op=OP.mult)
        lab_b = smallp.tile([128, NTILE], f32, tag="labb")
        nc.scalar.mul(lab_b[:], costm_b[:], SCALEC)
        nc.vector.tensor_tensor(out=lab_b[:], in0=lab_b[:], in1=valid_sb[:],
                                op=OP.mult)
        # ---- collective A: splits 0-2 partial sumexp + label terms ----
        cc_sb = smallp.tile([128, 2 * NTILE], f32, tag="ccsb")
        nc.vector.tensor_copy(out=cc_sb[:, NTILE:2 * NTILE], in_=lab_b[:])
        ps_s = pmisc.tile([1, N], f32, tag="misc")
        nc.tensor.matmul(out=ps_s[:], lhsT=ones_bf[:CTILE, :], rhs=acc[:],
                         start=True, stop=True)
        s_fm = smallp.tile([1, N], f32, tag="sfm")
        nc.vector.tensor_copy(out=s_fm[:], in_=ps_s[:])
        for j in range(NTILE):
            pst = pmisc.tile([128, 1], f32, tag="misc")
            nc.tensor.transpose(out=pst[:],
                                in_=s_fm[0:1, j * 128:(j + 1) * 128],
                                identity=ones_f[0:1, 0:1])
            nc.vector.tensor_copy(out=cc_sb[:, j:j + 1], in_=pst[:])
        nc.vector.tensor_tensor(out=cc_sb[:, 0:NTILE], in0=cc_sb[:, 0:NTILE],
                                in1=corr_b[:], op=OP.add)
        cc_in = dramp.tile([128, 2 * NTILE], f32, tag="ccin")
        cc_out = dramp.tile([128, 2 * NTILE], f32, tag="ccout")
        nc.sync.dma_start(out=cc_in[:], in_=cc_sb[:])
        nc.gpsimd.collective_compute(
            "AllReduce", mybir.AluOpType.add,
            replica_groups=[list(range(NCORES))],
            ins=[cc_in.opt()], outs=[cc_out.opt()])
        cc_res = smallp.tile([128, 2 * NTILE], f32, tag="ccres")
        nc.sync.dma_start(out=cc_res[:], in_=cc_out[:])

        emit_split(4, acc4)
        # ---- collective B: split 4's partial sumexp only ----
        ccb_sb = smallp.tile([128, NTILE], f32, tag="ccbsb")
        ps_s4 = pmisc.tile([1, N], f32, tag="misc")
        nc.tensor.matmul(out=ps_s4[:], lhsT=ones_bf[:CTILE, :], rhs=acc4[:],
                         start=True, stop=True)
        s_fm4 = smallp.tile([1, N], f32, tag="sfm4")
        nc.vector.tensor_copy(out=s_fm4[:], in_=ps_s4[:])
        for j in range(NTILE):
            pst = pmisc.tile([128, 1], f32, tag="misc")
            nc.tensor.transpose(out=pst[:],
                                in_=s_fm4[0:1, j * 128:(j + 1) * 128],
                                identity=ones_f[0:1, 0:1])
            nc.vector.tensor_copy(out=ccb_sb[:, j:j + 1], in_=pst[:])
        ccb_in = dramp.tile([128, NTILE], f32, tag="ccbin")
        ccb_out = dramp.tile([128, NTILE], f32, tag="ccbout")
        nc.sync.dma_start(out=ccb_in[:], in_=ccb_sb[:])
        nc.gpsimd.collective_compute(
            "AllReduce", mybir.AluOpType.add,
            replica_groups=[list(range(NCORES))],
            ins=[ccb_in.opt()], outs=[ccb_out.opt()])
        ccb_res = smallp.tile([128, NTILE], f32, tag="ccbres")
        nc.sync.dma_start(out=ccb_res[:], in_=ccb_out[:])

        sum_all = smallp.tile([128, NTILE], f32, tag="sumall")
        nc.vector.tensor_tensor(out=sum_all[:], in0=cc_res[:, 0:NTILE],
                                in1=ccb_res[:], op=OP.add)
        lse_b = smallp.tile([128, NTILE], f32, tag="lseb")
        nc.scalar.activation(lse_b[:], sum_all[:], AF.Ln, scale=EXP30)
        nc.vector.tensor_tensor(out=lse_b[:], in0=lse_b[:],
                                in1=cc_res[:, NTILE:2 * NTILE], op=OP.subtract)
        part = smallp.tile([128, 1], f32, tag="part")
        nc.vector.reduce_sum(out=part[:], in_=lse_b[:], axis=mybir.AxisListType.X)
        ps_l = pmisc.tile([1, 1], f32, tag="misc")
        nc.tensor.matmul(out=ps_l[:], lhsT=ones_f[:], rhs=part[:],
                         start=True, stop=True)
        loss_sb = smallp.tile([1, 1], f32, tag="loss")
        nc.scalar.mul(loss_sb[:], ps_l[:], 1.0 / N)
        nc.sync.dma_start(out=out_d[:, :], in_=loss_sb[:])

    nc.finalize()
    return nc


def _host_prep(embeddings, labels, weight):
    import ml_dtypes
    emb = np.ascontiguousarray(embeddings, dtype=np.float32)
    w = np.ascontiguousarray(weight, dtype=np.float32)
    lab = np.asarray(labels).astype(np.int64)
    ident_bf = np.eye(128, dtype=ml_dtypes.bfloat16)
    ident_f = np.eye(128, dtype=np.float32)
    ones_f = np.ones((128, 1), dtype=np.float32)
    ones_bf = np.ones((128, 1), dtype=ml_dtypes.bfloat16)
    ones_row = np.ones((1, 128), dtype=np.float32)
    in_maps = []
    for core in range(NCORES):
        sh = np.ascontiguousarray(w[core * CLOC:(core + 1) * CLOC])
        lab_loc = lab - core * CLOC
        valid = ((lab_loc >= 0) & (lab_loc < CLOC)).astype(np.float32)
        idx = np.clip(lab_loc, 0, CLOC - 1).astype(np.int32)
        in_maps.append({
            "wt": np.ascontiguousarray(sh.T) * np.float32(512.0),
            "wrows": sh,
            "emb": emb,
            "labidx": np.ascontiguousarray(idx.reshape(NTILE, 128).T),
            "valid": np.ascontiguousarray(valid.reshape(NTILE, 128).T),
            "identbf": ident_bf,
            "identf": ident_f,
            "onesf": ones_f,
            "onesbf": ones_bf,
            "onesrow": ones_row,
        })
    return in_maps


def run(embeddings, labels, weight, trace=False):
    from concourse import bass_utils
    if "nc" not in _cache:
        _cache["nc"] = _build()
    in_maps = _host_prep(embeddings, labels, weight)
    res = bass_utils.run_bass_kernel_spmd(
        _cache["nc"], in_maps, core_ids=list(range(NCORES)), trace=trace)
    out = np.asarray(res.results[0]["out"], dtype=np.float32).reshape(())
    return out, res


def kernel(embeddings, labels, weight):
    out, _ = run(embeddings, labels, weight, trace=False)
    return out



# revision 2
# speedup vs baseline: 1.5115x; 1.5115x over previous
"""AdaFace loss on 8 TRN2 NeuronCores — class-parallel margin softmax.

Sharding: class dim split 12500/core. Host pre-normalizes weight rows and
casts to fp8 in k-major DoubleRow layout [128, 2(kk), 2(o), 12500]; the
device streams W from HBM (6.4MB/core) into one big SBUF tile via chunked
DMAs. Matmul keeps the (transposed, normalized, fp8) embeddings stationary
and streams W: psum[128 batch, <=1536 classes] accumulates K=512 in 2
DoubleRow matmuls per 512-class sub-chunk. ACT then applies
exp(30*cos - 30) with accum_out producing per-sample sumexp partials
directly. Label-margin terms come from an f32 row gather and are combined
via an early AllReduce that hides under the main sweep; a warmup AllReduce
at t~0 absorbs core-start skew; the final sumexp AllReduce is the only
exposed collective.
"""
import math
import numpy as np

NCORES = 8
C, D, N = 100000, 512, 512
CLOC = C // NCORES            # 12500
SUB = 512                     # classes per matmul / psum bank
GRP = 3 * SUB                 # classes per psum tile / ACT exp (3 banks)
NGRP_FULL = CLOC // GRP       # 8 full groups
GRP_LAST = CLOC - NGRP_FULL * GRP   # 212
NGRP = NGRP_FULL + 1          # 9
NTILE = N // 128              # 4 batch tiles
SCALEC = 30.0
MARGIN = 0.4
HCONST = 0.333
FP8S = 16.0                   # fp8 scaling for both operands
S30 = SCALEC / (FP8S * FP8S)  # activation scale: psum = 256*cos
EXP30 = float(np.exp(np.float32(30.0)))

_cache = {}


def _build():
    import concourse.bass as bass
    import concourse.bacc as bacc
    import concourse.mybir as mybir
    import concourse.tile as tile
    from contextlib import ExitStack

    f32 = mybir.dt.float32
    bf16 = mybir.dt.bfloat16
    fp8 = mybir.dt.float8e4
    i32 = mybir.dt.int32
    AF = mybir.ActivationFunctionType
    OP = mybir.AluOpType

    nc = bacc.Bacc("TRN2", target_bir_lowering=False, debug=False,
                   num_devices=NCORES)
    _c30 = nc.alloc_sbuf_tensor("const-f32-neg30", [128, 1], f32)
    nc.gpsimd.memset(_c30.ap(), -30.0)
    nc.const_aps.aps[(f32, -30.0)] = _c30.ap()
    nc.all_engine_barrier()

    wt8_d = nc.dram_tensor("wt8", [128, 2, 2, CLOC], fp8, kind="ExternalInput")
    wrows_d = nc.dram_tensor("wrows", [CLOC, D], f32, kind="ExternalInput")
    emb_d = nc.dram_tensor("emb", [N, D], f32, kind="ExternalInput")
    labidx_d = nc.dram_tensor("labidx", [128, NTILE], i32, kind="ExternalInput")
    valid_d = nc.dram_tensor("valid", [128, NTILE], f32, kind="ExternalInput")
    identbf_d = nc.dram_tensor("identbf", [128, 128], bf16, kind="ExternalInput")
    onesf_d = nc.dram_tensor("onesf", [128, 1], f32, kind="ExternalInput")
    onesrow_d = nc.dram_tensor("onesrow", [1, 128], f32, kind="ExternalInput")
    out_d = nc.dram_tensor("out", [1, 1], f32, kind="ExternalOutput")

    with tile.TileContext(nc) as tc, ExitStack() as ctx:
        constp = ctx.enter_context(tc.tile_pool(name="const", bufs=1))
        scrp = ctx.enter_context(tc.tile_pool(name="scratch", bufs=2))
        actp = ctx.enter_context(tc.tile_pool(name="actout", bufs=2))
        smallp = ctx.enter_context(tc.tile_pool(name="small", bufs=2))
        pmain = ctx.enter_context(tc.tile_pool(name="pmain", bufs=2, space="PSUM"))
        pmisc = ctx.enter_context(tc.tile_pool(name="pmisc", bufs=2, space="PSUM"))
        dramp = ctx.enter_context(tc.tile_pool(name="dram", bufs=1, space="DRAM"))

        # ---- warmup collective: absorb core-start skew under the DMA phase
        warm_sb = smallp.tile([128, 1], f32, tag="warm")
        nc.vector.memset(warm_sb[:], 0.0)
        warm_in = dramp.tile([128, 1], f32, tag="warmin")
        warm_out = dramp.tile([128, 1], f32, tag="warmout")
        nc.sync.dma_start(out=warm_in[:], in_=warm_sb[:])
        nc.gpsimd.collective_compute(
            "AllReduce", mybir.AluOpType.add,
            replica_groups=[list(range(NCORES))],
            ins=[warm_in.opt()], outs=[warm_out.opt()])

        # ---- weight shard: one big SBUF tile, chunked group-aligned DMAs
        w8 = constp.tile([128, 2, 2, CLOC], fp8, tag="w8")
        for g in range(NGRP):
            c0 = g * GRP
            w = GRP if g < NGRP_FULL else GRP_LAST
            nc.gpsimd.dma_start(out=w8[:, :, :, c0:c0 + w],
                                in_=wt8_d[:, :, :, c0:c0 + w])

        # ---- constants ----
        ident_bf = constp.tile([128, 128], bf16, tag="identbf")
        nc.sync.dma_start(out=ident_bf[:], in_=identbf_d[:, :])
        ones_f = constp.tile([128, 1], f32, tag="onesf")
        nc.sync.dma_start(out=ones_f[:], in_=onesf_d[:, :])
        ones_row = constp.tile([1, 128], f32, tag="onesrow")
        nc.sync.dma_start(out=ones_row[:], in_=onesrow_d[:, :])
        idx_sb = constp.tile([128, NTILE], i32, tag="idx")
        nc.sync.dma_start(out=idx_sb[:], in_=labidx_d[:, :])
        valid_sb = constp.tile([128, NTILE], f32, tag="valid")
        nc.sync.dma_start(out=valid_sb[:], in_=valid_d[:, :])

        def rsqrt(x_ap, y_ap, t_ap, iters=2):
            """y = 1/sqrt(x) via bitcast seed + Newton (x > 0)."""
            xi = x_ap.bitcast(i32)
            yi = y_ap.bitcast(i32)
            nc.vector.tensor_scalar(out=yi, in0=xi, scalar1=1, scalar2=None,
                                    op0=OP.arith_shift_right)
            nc.vector.tensor_scalar(out=yi, in0=yi, scalar1=-1,
                                    scalar2=0x5f3759df, op0=OP.mult, op1=OP.add)
            for _ in range(iters):
                nc.vector.tensor_tensor(out=t_ap, in0=x_ap, in1=y_ap, op=OP.mult)
                nc.vector.tensor_tensor(out=t_ap, in0=t_ap, in1=y_ap, op=OP.mult)
                nc.vector.tensor_scalar(out=t_ap, in0=t_ap, scalar1=-0.5,
                                        scalar2=1.5, op0=OP.mult, op1=OP.add)
                nc.vector.tensor_tensor(out=y_ap, in0=y_ap, in1=t_ap, op=OP.mult)

        # ---- label-row gather (early: overlaps weight DMA) ----
        wlab_t = []
        for j in range(NTILE):
            wl = constp.tile([128, D], f32, tag=f"wlab{j}")
            nc.gpsimd.indirect_dma_start(
                out=wl[:], out_offset=None, in_=wrows_d[:, :],
                in_offset=bass.IndirectOffsetOnAxis(ap=idx_sb[:, j:j + 1],
                                                    axis=0))
            wlab_t.append(wl)

        # ---- embedding prep ----
        emb_t = []
        for j in range(NTILE):
            t = constp.tile([128, D], f32, tag=f"emb{j}")
            nc.sync.dma_start(out=t[:], in_=emb_d[j * 128:(j + 1) * 128, :])
            emb_t.append(t)

        norms2_b = constp.tile([128, NTILE], f32, tag="norms2")
        for j in range(NTILE):
            scr = scrp.tile([128, D], f32, tag="sq")
            nc.vector.scalar_tensor_tensor(
                out=scr[:], in0=emb_t[j][:], scalar=1.0, in1=emb_t[j][:],
                op0=OP.mult, op1=OP.mult, accum_out=norms2_b[:, j:j + 1])
        invn_b = constp.tile([128, NTILE], f32, tag="invn")
        tmp_b = scrp.tile([128, NTILE], f32, tag="tmpb")
        rsqrt(norms2_b[:], invn_b[:], tmp_b[:])                # 1/||e||
        norms_b = constp.tile([128, NTILE], f32, tag="norms")
        nc.vector.tensor_tensor(out=norms_b[:], in0=norms2_b[:], in1=invn_b[:],
                                op=OP.mult)                    # ||e||

        # normalize, scale x16 to bf16, transpose into fp8 embT8[kk][p,o,n]
        embT8 = [constp.tile([128, 2, N], fp8, name=f"embT8_{kk}",
                             tag=f"embT8_{kk}") for kk in range(2)]
        for j in range(NTILE):
            nc.vector.tensor_scalar_mul(emb_t[j][:], emb_t[j][:],
                                        invn_b[:, j:j + 1])
            e16 = scrp.tile([128, D], bf16, tag="e16")
            nc.vector.tensor_scalar(out=e16[:], in0=emb_t[j][:],
                                    scalar1=FP8S, scalar2=None, op0=OP.mult)
            for k4 in range(4):
                pst = pmisc.tile([128, 128], bf16, tag="misc")
                nc.tensor.transpose(out=pst[:],
                                    in_=e16[:, k4 * 128:(k4 + 1) * 128],
                                    identity=ident_bf[:])
                nc.vector.tensor_copy(
                    out=embT8[k4 // 2][:, k4 % 2, j * 128:(j + 1) * 128],
                    in_=pst[:])

        # ---- batch stats: mean/std of ||e|| ----
        stat_in = smallp.tile([128, 2 * NTILE], f32, tag="statin")
        nc.vector.tensor_copy(out=stat_in[:, 0:NTILE], in_=norms_b[:])
        nc.vector.tensor_copy(out=stat_in[:, NTILE:2 * NTILE], in_=norms2_b[:])
        ps_stat = pmisc.tile([1, 2 * NTILE], f32, tag="misc")
        nc.tensor.matmul(out=ps_stat[:], lhsT=ones_f[:], rhs=stat_in[:],
                         start=True, stop=True)
        sums2 = smallp.tile([1, 2], f32, tag="sums2")
        nc.vector.reduce_sum(out=sums2[:, 0:1], in_=ps_stat[:, 0:NTILE],
                             axis=mybir.AxisListType.X)
        nc.vector.reduce_sum(out=sums2[:, 1:2], in_=ps_stat[:, NTILE:2 * NTILE],
                             axis=mybir.AxisListType.X)
        # mean = S1/N ; var = (S2 - S1^2/N)/(N-1); std = sqrt(var)
        scal = smallp.tile([1, 4], f32, tag="scal")
        nc.scalar.mul(scal[:, 0:1], sums2[:, 0:1], 1.0 / N)           # mean
        nc.vector.tensor_tensor(out=scal[:, 1:2], in0=sums2[:, 0:1],
                                in1=sums2[:, 0:1], op=OP.mult)        # S1^2
        nc.scalar.mul(scal[:, 1:2], scal[:, 1:2], 1.0 / N)            # S1^2/N
        nc.vector.tensor_tensor(out=scal[:, 1:2], in0=sums2[:, 1:2],
                                in1=scal[:, 1:2], op=OP.subtract)
        nc.scalar.mul(scal[:, 1:2], scal[:, 1:2], 1.0 / (N - 1))      # var
        sct = smallp.tile([1, 2], f32, tag="sct")
        rsqrt(scal[:, 1:2], scal[:, 2:3], sct[:, 0:1])                # 1/std
        nc.vector.tensor_tensor(out=scal[:, 2:3], in0=scal[:, 1:2],
                                in1=scal[:, 2:3], op=OP.mult)         # std
        nc.vector.tensor_scalar_add(scal[:, 2:3], scal[:, 2:3], HCONST)  # std+H
        nc.vector.reciprocal(out=scal[:, 3:4], in_=scal[:, 2:3])      # 1/(std+H)
        # broadcast mean, 1/(std+H) to 128 partitions via K=1 outer product
        bvals = smallp.tile([1, 2], f32, tag="bvals")
        nc.vector.tensor_copy(out=bvals[:, 0:1], in_=scal[:, 0:1])
        nc.vector.tensor_copy(out=bvals[:, 1:2], in_=scal[:, 3:4])
        ps_bc = pmisc.tile([128, 2], f32, tag="misc")
        nc.tensor.matmul(out=ps_bc[:], lhsT=ones_row[:], rhs=bvals[:],
                         start=True, stop=True)
        bc = smallp.tile([128, 2], f32, tag="bc")
        nc.vector.tensor_copy(out=bc[:], in_=ps_bc[:])

        # margin scaler -> m, sin(m), cos(m) (poly, avoids Sin table set)
        ms_b = smallp.tile([128, NTILE], f32, tag="msb")
        nc.vector.tensor_tensor(out=ms_b[:], in0=norms_b[:],
                                in1=bc[:, 0:1].to_broadcast([128, NTILE]),
                                op=OP.subtract)
        nc.vector.tensor_tensor(out=ms_b[:], in0=ms_b[:],
                                in1=bc[:, 1:2].to_broadcast([128, NTILE]),
                                op=OP.mult)
        nc.vector.tensor_scalar_min(ms_b[:], ms_b[:], 1.0)
        nc.vector.tensor_scalar_max(ms_b[:], ms_b[:], -1.0)
        m_b = smallp.tile([128, NTILE], f32, tag="mb")
        nc.vector.tensor_scalar(out=m_b[:], in0=ms_b[:], scalar1=MARGIN,
                                scalar2=MARGIN, op0=OP.mult, op1=OP.add)
        u_b = smallp.tile([128, NTILE], f32, tag="ub")
        nc.vector.tensor_tensor(out=u_b[:], in0=m_b[:], in1=m_b[:], op=OP.mult)
        sin_b = smallp.tile([128, NTILE], f32, tag="sinb")
        nc.vector.tensor_scalar(out=sin_b[:], in0=u_b[:], scalar1=1.0 / 120,
                                scalar2=-1.0 / 6, op0=OP.mult, op1=OP.add)
        nc.vector.tensor_tensor(out=sin_b[:], in0=sin_b[:], in1=u_b[:], op=OP.mult)
        nc.vector.tensor_scalar_add(sin_b[:], sin_b[:], 1.0)
        nc.vector.tensor_tensor(out=sin_b[:], in0=sin_b[:], in1=m_b[:], op=OP.mult)
        cos_b = smallp.tile([128, NTILE], f32, tag="cosb")
        nc.vector.tensor_scalar(out=cos_b[:], in0=u_b[:], scalar1=-1.0 / 720,
                                scalar2=1.0 / 24, op0=OP.mult, op1=OP.add)
        nc.vector.tensor_tensor(out=cos_b[:], in0=cos_b[:], in1=u_b[:], op=OP.mult)
        nc.vector.tensor_scalar_add(cos_b[:], cos_b[:], -0.5)
        nc.vector.tensor_tensor(out=cos_b[:], in0=cos_b[:], in1=u_b[:], op=OP.mult)
        nc.vector.tensor_scalar_add(cos_b[:], cos_b[:], 1.0)

        # ---- label path: cos_t from normalized gather rows (unit norm) ----
        dots_b = smallp.tile([128, NTILE], f32, tag="dots")
        for j in range(NTILE):
            scr2 = scrp.tile([128, D], f32, tag="sq")
            nc.vector.scalar_tensor_tensor(
                out=scr2[:], in0=emb_t[j][:], scalar=1.0, in1=wlab_t[j][:],
                op0=OP.mult, op1=OP.mult, accum_out=dots_b[:, j:j + 1])
        cost_b = smallp.tile([128, NTILE], f32, tag="cost")
        nc.vector.tensor_scalar_min(cost_b[:], dots_b[:], 1.0)
        nc.vector.tensor_scalar_max(cost_b[:], cost_b[:], -1.0)
        q_b = smallp.tile([128, NTILE], f32, tag="qb")
        nc.vector.tensor_tensor(out=q_b[:], in0=cost_b[:], in1=cost_b[:],
                                op=OP.mult)
        # rt = sqrt(1 - cos^2) = q*rsqrt(q) with q clamped away from 0
        nc.vector.tensor_scalar(out=q_b[:], in0=q_b[:], scalar1=-1.0,
                                scalar2=1.0, op0=OP.mult, op1=OP.add)
        nc.vector.tensor_scalar_max(q_b[:], q_b[:], 1e-12)
        rt_b = smallp.tile([128, NTILE], f32, tag="rtb")
        tmp3_b = scrp.tile([128, NTILE], f32, tag="tmpb")
        rsqrt(q_b[:], rt_b[:], tmp3_b[:])
        nc.vector.tensor_tensor(out=rt_b[:], in0=rt_b[:], in1=q_b[:],
                                op=OP.mult)
        costm_b = smallp.tile([128, NTILE], f32, tag="costm")
        nc.vector.tensor_tensor(out=costm_b[:], in0=cost_b[:], in1=cos_b[:],
                                op=OP.mult)
        nc.vector.tensor_tensor(out=rt_b[:], in0=rt_b[:], in1=sin_b[:],
                                op=OP.mult)
        nc.vector.tensor_tensor(out=costm_b[:], in0=costm_b[:], in1=rt_b[:],
                                op=OP.subtract)
        et_b = smallp.tile([128, NTILE], f32, tag="etb")
        nc.scalar.activation(et_b[:], cost_b[:], AF.Exp, bias=-30.0, scale=SCALEC)
        em_b = smallp.tile([128, NTILE], f32, tag="emb2")
        nc.scalar.activation(em_b[:], costm_b[:], AF.Exp, bias=-30.0, scale=SCALEC)
        corr_b = smallp.tile([128, NTILE], f32, tag="corrb")
        nc.vector.tensor_tensor(out=corr_b[:], in0=em_b[:], in1=et_b[:],
                                op=OP.subtract)
        nc.vector.tensor_tensor(out=corr_b[:], in0=corr_b[:], in1=valid_sb[:],
                                op=OP.mult)
        lab_b = smallp.tile([128, NTILE], f32, tag="labb")
        nc.scalar.mul(lab_b[:], costm_b[:], SCALEC)
        nc.vector.tensor_tensor(out=lab_b[:], in0=lab_b[:], in1=valid_sb[:],
                                op=OP.mult)

        # ---- collective 1: label corr + label logits (hides under sweep) ----
        cc1_sb = smallp.tile([128, 2 * NTILE], f32, tag="cc1sb")
        nc.vector.tensor_copy(out=cc1_sb[:, 0:NTILE], in_=corr_b[:])
        nc.vector.tensor_copy(out=cc1_sb[:, NTILE:2 * NTILE], in_=lab_b[:])
        cc1_in = dramp.tile([128, 2 * NTILE], f32, tag="cc1in")
        cc1_out = dramp.tile([128, 2 * NTILE], f32, tag="cc1out")
        nc.sync.dma_start(out=cc1_in[:], in_=cc1_sb[:])
        nc.gpsimd.collective_compute(
            "AllReduce", mybir.AluOpType.add,
            replica_groups=[list(range(NCORES))],
            ins=[cc1_in.opt()], outs=[cc1_out.opt()])
        cc1_res = smallp.tile([128, 2 * NTILE], f32, tag="cc1res")
        nc.sync.dma_start(out=cc1_res[:], in_=cc1_out[:])

        # ---- main sweep: g outer (DMA streaming order), j inner ----
        sums = constp.tile([128, NTILE * NGRP], f32, tag="sums")
        for g in range(NGRP):
            c0 = g * GRP
            w = GRP if g < NGRP_FULL else GRP_LAST
            nsub = (w + SUB - 1) // SUB
            for j in range(NTILE):
                ps = pmain.tile([128, GRP], f32, tag="ps")
                for kk in range(2):
                    for s in range(nsub):
                        ws = min(SUB, w - s * SUB)
                        nc.tensor.matmul(
                            out=ps[:, s * SUB:s * SUB + ws],
                            lhsT=embT8[kk][:, :, j * 128:(j + 1) * 128],
                            rhs=w8[:, kk, :, c0 + s * SUB:c0 + s * SUB + ws],
                            perf_mode=mybir.MatmulPerfMode.DoubleRow,
                            start=(kk == 0), stop=(kk == 1))
                ex = actp.tile([128, GRP], bf16, tag="ex")
                nc.scalar.activation(ex[:, 0:w], ps[:, 0:w], AF.Exp,
                                     bias=-30.0, scale=S30,
                                     accum_out=sums[:, j * NGRP + g:
                                                    j * NGRP + g + 1])

        # ---- per-sample totals + final collective ----
        stot = smallp.tile([128, NTILE], f32, tag="stot")
        for j in range(NTILE):
            nc.vector.reduce_sum(out=stot[:, j:j + 1],
                                 in_=sums[:, j * NGRP:(j + 1) * NGRP],
                                 axis=mybir.AxisListType.X)
        cc2_in = dramp.tile([128, NTILE], f32, tag="cc2in")
        cc2_out = dramp.tile([128, NTILE], f32, tag="cc2out")
        nc.sync.dma_start(out=cc2_in[:], in_=stot[:])
        nc.gpsimd.collective_compute(
            "AllReduce", mybir.AluOpType.add,
            replica_groups=[list(range(NCORES))],
            ins=[cc2_in.opt()], outs=[cc2_out.opt()])
        cc2_res = smallp.tile([128, NTILE], f32, tag="cc2res")
        nc.sync.dma_start(out=cc2_res[:], in_=cc2_out[:])

        sum_all = smallp.tile([128, NTILE], f32, tag="sumall")
        nc.vector.tensor_tensor(out=sum_all[:], in0=cc2_res[:],
                                in1=cc1_res[:, 0:NTILE], op=OP.add)
        lse_b = smallp.tile([128, NTILE], f32, tag="lseb")
        nc.scalar.activation(lse_b[:], sum_all[:], AF.Ln, scale=EXP30)
        nc.vector.tensor_tensor(out=lse_b[:], in0=lse_b[:],
                                in1=cc1_res[:, NTILE:2 * NTILE], op=OP.subtract)
        part = smallp.tile([128, 1], f32, tag="part")
        nc.vector.reduce_sum(out=part[:], in_=lse_b[:], axis=mybir.AxisListType.X)
        ps_l = pmisc.tile([1, 1], f32, tag="misc")
        nc.tensor.matmul(out=ps_l[:], lhsT=ones_f[:], rhs=part[:],
                         start=True, stop=True)
        loss_sb = smallp.tile([1, 1], f32, tag="loss")
        nc.scalar.mul(loss_sb[:], ps_l[:], 1.0 / N)
        nc.sync.dma_start(out=out_d[:, :], in_=loss_sb[:])

    nc.finalize()
    return nc


def _host_prep(embeddings, labels, weight):
    import ml_dtypes
    emb = np.ascontiguousarray(embeddings, dtype=np.float32)
    w = np.ascontiguousarray(weight, dtype=np.float32)
    lab = np.asarray(labels).astype(np.int64)
    # normalize rows once for the full weight matrix
    wn = np.sqrt((w * w).sum(axis=1, keepdims=True))
    wu = w / wn
    # k-major fp8 layout for the whole matrix: [128(p), 2(kk), 2(o), C]
    # with k = kk*256 + o*128 + p
    wt8_full = np.ascontiguousarray(
        (wu.T * np.float32(FP8S)).reshape(2, 2, 128, C).transpose(2, 0, 1, 3)
    ).astype(ml_dtypes.float8_e4m3)
    ident_bf = np.eye(128, dtype=ml_dtypes.bfloat16)
    ones_f = np.ones((128, 1), dtype=np.float32)
    ones_row = np.ones((1, 128), dtype=np.float32)
    in_maps = []
    for core in range(NCORES):
        lab_loc = lab - core * CLOC
        valid = ((lab_loc >= 0) & (lab_loc < CLOC)).astype(np.float32)
        idx = np.clip(lab_loc, 0, CLOC - 1).astype(np.int32)
        in_maps.append({
            "wt8": np.ascontiguousarray(
                wt8_full[:, :, :, core * CLOC:(core + 1) * CLOC]),
            "wrows": np.ascontiguousarray(wu[core * CLOC:(core + 1) * CLOC]),
            "emb": emb,
            "labidx": np.ascontiguousarray(idx.reshape(NTILE, 128).T),
            "valid": np.ascontiguousarray(valid.reshape(NTILE, 128).T),
            "identbf": ident_bf,
            "onesf": ones_f,
            "onesrow": ones_row,
        })
    return in_maps


def run(embeddings, labels, weight, trace=False):
    from concourse import bass_utils
    if "nc" not in _cache:
        _cache["nc"] = _build()
    in_maps = _host_prep(embeddings, labels, weight)
    res = bass_utils.run_bass_kernel_spmd(
        _cache["nc"], in_maps, core_ids=list(range(NCORES)), trace=trace)
    out = np.asarray(res.results[0]["out"], dtype=np.float32).reshape(())
    return out, res


def kernel(embeddings, labels, weight):
    out, _ = run(embeddings, labels, weight, trace=False)
    return out


# revision 6
# speedup vs baseline: 1.6848x; 1.1147x over previous
"""AdaFace loss on 8 TRN2 NeuronCores — class-parallel margin softmax.

Sharding: class dim split 12500/core. Host pre-normalizes weight rows and
casts to fp8 in k-major DoubleRow layout [128, 2(kk), 2(o), 12500]; the
device streams W from HBM (6.4MB/core) into one big SBUF tile via chunked
DMAs. Matmul keeps the (transposed, normalized, fp8) embeddings stationary
and streams W: psum[128 batch, <=1536 classes] accumulates K=512 in 2
DoubleRow matmuls per 512-class sub-chunk. ACT then applies
exp(30*cos - 30) with accum_out producing per-sample sumexp partials
directly. Label-margin terms come from an f32 row gather and are combined
via an early AllReduce that hides under the main sweep; a warmup AllReduce
at t~0 absorbs core-start skew; the final sumexp AllReduce is the only
exposed collective.
"""
import math
import numpy as np

NCORES = 8
C, D, N = 100000, 512, 512
CLOC = C // NCORES            # 12500
SUB = 512                     # classes per matmul / psum bank
GRP = 3 * SUB                 # classes per psum tile / ACT exp (3 banks)
NGRP_FULL = CLOC // GRP       # 8 full groups
GRP_LAST = CLOC - NGRP_FULL * GRP   # 212
NGRP = NGRP_FULL + 1          # 9
NTILE = N // 128              # 4 batch tiles
SCALEC = 30.0
MARGIN = 0.4
HCONST = 0.333
FP8S = 16.0                   # fp8 scaling for both operands
S30 = SCALEC / (FP8S * FP8S)  # activation scale: psum = 256*cos
EXP30 = float(np.exp(np.float32(30.0)))
# Schraudolph fast-exp: exp(y) ~= bitcast_f32(int(y*FEA + FEB)); for the
# DVE-offloaded groups y = S30*psum - 30, so i = psum*(FEA*S30) + (FEB - 30*FEA)
FEA = 12102203.161561485      # 2^23/ln(2)
FEB = 1064866805.0
DVE_A = FEA * S30
DVE_B = FEB - 30.0 * FEA

_cache = {}


def _build():
    import concourse.bass as bass
    import concourse.bacc as bacc
    import concourse.mybir as mybir
    import concourse.tile as tile
    from contextlib import ExitStack

    f32 = mybir.dt.float32
    bf16 = mybir.dt.bfloat16
    fp8 = mybir.dt.float8e4
    i32 = mybir.dt.int32
    AF = mybir.ActivationFunctionType
    OP = mybir.AluOpType

    nc = bacc.Bacc("TRN2", target_bir_lowering=False, debug=False,
                   num_devices=NCORES)
    _c30 = nc.alloc_sbuf_tensor("const-f32-neg30", [128, 1], f32)
    nc.gpsimd.memset(_c30.ap(), -30.0)
    nc.const_aps.aps[(f32, -30.0)] = _c30.ap()
    nc.all_engine_barrier()

    wt8_d = nc.dram_tensor("wt8", [128, 2, 2, CLOC], fp8, kind="ExternalInput")
    wrows_d = nc.dram_tensor("wrows", [CLOC, D], f32, kind="ExternalInput")
    emb_d = nc.dram_tensor("emb", [N, D], f32, kind="ExternalInput")
    labidx_d = nc.dram_tensor("labidx", [128, NTILE], i32, kind="ExternalInput")
    valid_d = nc.dram_tensor("valid", [128, NTILE], f32, kind="ExternalInput")
    identbf_d = nc.dram_tensor("identbf", [128, 128], bf16, kind="ExternalInput")
    onesf_d = nc.dram_tensor("onesf", [128, 1], f32, kind="ExternalInput")
    onesrow_d = nc.dram_tensor("onesrow", [1, 128], f32, kind="ExternalInput")
    out_d = nc.dram_tensor("out", [1, 1], f32, kind="ExternalOutput")

    with tile.TileContext(nc) as tc, ExitStack() as ctx:
        constp = ctx.enter_context(tc.tile_pool(name="const", bufs=1))
        scrp = ctx.enter_context(tc.tile_pool(name="scratch", bufs=2))
        actp = ctx.enter_context(tc.tile_pool(name="actout", bufs=2))
        smallp = ctx.enter_context(tc.tile_pool(name="small", bufs=2))
        pmain = ctx.enter_context(tc.tile_pool(name="pmain", bufs=2, space="PSUM"))
        pmisc = ctx.enter_context(tc.tile_pool(name="pmisc", bufs=2, space="PSUM"))
        dramp = ctx.enter_context(tc.tile_pool(name="dram", bufs=1, space="DRAM"))

        # ---- weight shard: one big SBUF tile, chunked group-aligned DMAs.
        # Issued on the Scalar engine (idle at t=0; gpsimd is stuck in init
        # for ~10us) so HBM streaming starts immediately.
        w8 = constp.tile([128, 2, 2, CLOC], fp8, tag="w8")
        for g in range(NGRP):
            c0 = g * GRP
            w = GRP if g < NGRP_FULL else GRP_LAST
            nc.scalar.dma_start(out=w8[:, :, :, c0:c0 + w],
                                in_=wt8_d[:, :, :, c0:c0 + w])

        # ---- embeddings first on the sync queue (gate the prep chain) ----
        emb_t = []
        for j in range(NTILE):
            t = constp.tile([128, D], f32, tag=f"emb{j}")
            nc.sync.dma_start(out=t[:], in_=emb_d[j * 128:(j + 1) * 128, :])
            emb_t.append(t)

        # ---- constants ----
        ident_bf = constp.tile([128, 128], bf16, tag="identbf")
        nc.sync.dma_start(out=ident_bf[:], in_=identbf_d[:, :])
        ones_f = constp.tile([128, 1], f32, tag="onesf")
        nc.sync.dma_start(out=ones_f[:], in_=onesf_d[:, :])
        ones_row = constp.tile([1, 128], f32, tag="onesrow")
        nc.sync.dma_start(out=ones_row[:], in_=onesrow_d[:, :])
        idx_sb = constp.tile([128, NTILE], i32, tag="idx")
        nc.sync.dma_start(out=idx_sb[:], in_=labidx_d[:, :])
        valid_sb = constp.tile([128, NTILE], f32, tag="valid")
        nc.sync.dma_start(out=valid_sb[:], in_=valid_d[:, :])

        def rsqrt(x_ap, y_ap, t_ap, iters=2):
            """y = 1/sqrt(x) via bitcast seed + Newton (x > 0)."""
            xi = x_ap.bitcast(i32)
            yi = y_ap.bitcast(i32)
            nc.vector.tensor_scalar(out=yi, in0=xi, scalar1=1, scalar2=None,
                                    op0=OP.arith_shift_right)
            nc.vector.tensor_scalar(out=yi, in0=yi, scalar1=-1,
                                    scalar2=0x5f3759df, op0=OP.mult, op1=OP.add)
            for _ in range(iters):
                nc.vector.tensor_tensor(out=t_ap, in0=x_ap, in1=y_ap, op=OP.mult)
                nc.vector.tensor_tensor(out=t_ap, in0=t_ap, in1=y_ap, op=OP.mult)
                nc.vector.tensor_scalar(out=t_ap, in0=t_ap, scalar1=-0.5,
                                        scalar2=1.5, op0=OP.mult, op1=OP.add)
                nc.vector.tensor_tensor(out=y_ap, in0=y_ap, in1=t_ap, op=OP.mult)

        # ---- label-row gather (early: overlaps weight DMA) ----
        wlab_t = []
        for j in range(NTILE):
            wl = constp.tile([128, D], f32, tag=f"wlab{j}")
            nc.gpsimd.indirect_dma_start(
                out=wl[:], out_offset=None, in_=wrows_d[:, :],
                in_offset=bass.IndirectOffsetOnAxis(ap=idx_sb[:, j:j + 1],
                                                    axis=0))
            wlab_t.append(wl)

        # ---- embedding prep ----
        norms2_b = constp.tile([128, NTILE], f32, tag="norms2")
        for j in range(NTILE):
            scr = scrp.tile([128, D], f32, tag="sq")
            nc.vector.scalar_tensor_tensor(
                out=scr[:], in0=emb_t[j][:], scalar=1.0, in1=emb_t[j][:],
                op0=OP.mult, op1=OP.mult, accum_out=norms2_b[:, j:j + 1])
        invn_b = constp.tile([128, NTILE], f32, tag="invn")
        tmp_b = scrp.tile([128, NTILE], f32, tag="tmpb")
        rsqrt(norms2_b[:], invn_b[:], tmp_b[:])                # 1/||e||
        norms_b = constp.tile([128, NTILE], f32, tag="norms")
        nc.vector.tensor_tensor(out=norms_b[:], in0=norms2_b[:], in1=invn_b[:],
                                op=OP.mult)                    # ||e||

        # normalize, scale x16 to bf16, transpose into fp8 embT8[kk][p,o,n]
        embT8 = [constp.tile([128, 2, N], fp8, name=f"embT8_{kk}",
                             tag=f"embT8_{kk}") for kk in range(2)]
        for j in range(NTILE):
            nc.vector.tensor_scalar_mul(emb_t[j][:], emb_t[j][:],
                                        invn_b[:, j:j + 1])
            e16 = scrp.tile([128, D], bf16, tag="e16")
            nc.vector.tensor_scalar(out=e16[:], in0=emb_t[j][:],
                                    scalar1=FP8S, scalar2=None, op0=OP.mult)
            for k4 in range(4):
                pst = pmisc.tile([128, 128], bf16, tag="misc")
                nc.tensor.transpose(out=pst[:],
                                    in_=e16[:, k4 * 128:(k4 + 1) * 128],
                                    identity=ident_bf[:])
                nc.vector.tensor_copy(
                    out=embT8[k4 // 2][:, k4 % 2, j * 128:(j + 1) * 128],
                    in_=pst[:])

        # ---- batch stats: mean/std of ||e|| ----
        stat_in = smallp.tile([128, 2 * NTILE], f32, tag="statin")
        nc.vector.tensor_copy(out=stat_in[:, 0:NTILE], in_=norms_b[:])
        nc.vector.tensor_copy(out=stat_in[:, NTILE:2 * NTILE], in_=norms2_b[:])
        ps_stat = pmisc.tile([1, 2 * NTILE], f32, tag="misc")
        nc.tensor.matmul(out=ps_stat[:], lhsT=ones_f[:], rhs=stat_in[:],
                         start=True, stop=True)
        sums2 = smallp.tile([1, 2], f32, tag="sums2")
        nc.vector.reduce_sum(out=sums2[:, 0:1], in_=ps_stat[:, 0:NTILE],
                             axis=mybir.AxisListType.X)
        nc.vector.reduce_sum(out=sums2[:, 1:2], in_=ps_stat[:, NTILE:2 * NTILE],
                             axis=mybir.AxisListType.X)
        # mean = S1/N ; var = (S2 - S1^2/N)/(N-1); std = sqrt(var)
        scal = smallp.tile([1, 4], f32, tag="scal")
        nc.scalar.mul(scal[:, 0:1], sums2[:, 0:1], 1.0 / N)           # mean
        nc.vector.tensor_tensor(out=scal[:, 1:2], in0=sums2[:, 0:1],
                                in1=sums2[:, 0:1], op=OP.mult)        # S1^2
        nc.scalar.mul(scal[:, 1:2], scal[:, 1:2], 1.0 / N)            # S1^2/N
        nc.vector.tensor_tensor(out=scal[:, 1:2], in0=sums2[:, 1:2],
                                in1=scal[:, 1:2], op=OP.subtract)
        nc.scalar.mul(scal[:, 1:2], scal[:, 1:2], 1.0 / (N - 1))      # var
        sct = smallp.tile([1, 2], f32, tag="sct")
        rsqrt(scal[:, 1:2], scal[:, 2:3], sct[:, 0:1])                # 1/std
        nc.vector.tensor_tensor(out=scal[:, 2:3], in0=scal[:, 1:2],
                                in1=scal[:, 2:3], op=OP.mult)         # std
        nc.vector.tensor_scalar_add(scal[:, 2:3], scal[:, 2:3], HCONST)  # std+H
        nc.vector.reciprocal(out=scal[:, 3:4], in_=scal[:, 2:3])      # 1/(std+H)
        # broadcast mean, 1/(std+H) to 128 partitions via K=1 outer product
        bvals = smallp.tile([1, 2], f32, tag="bvals")
        nc.vector.tensor_copy(out=bvals[:, 0:1], in_=scal[:, 0:1])
        nc.vector.tensor_copy(out=bvals[:, 1:2], in_=scal[:, 3:4])
        ps_bc = pmisc.tile([128, 2], f32, tag="misc")
        nc.tensor.matmul(out=ps_bc[:], lhsT=ones_row[:], rhs=bvals[:],
                         start=True, stop=True)
        bc = smallp.tile([128, 2], f32, tag="bc")
        nc.vector.tensor_copy(out=bc[:], in_=ps_bc[:])

        # margin scaler -> m, sin(m), cos(m) (poly, avoids Sin table set)
        ms_b = smallp.tile([128, NTILE], f32, tag="msb")
        nc.vector.tensor_tensor(out=ms_b[:], in0=norms_b[:],
                                in1=bc[:, 0:1].to_broadcast([128, NTILE]),
                                op=OP.subtract)
        nc.vector.tensor_tensor(out=ms_b[:], in0=ms_b[:],
                                in1=bc[:, 1:2].to_broadcast([128, NTILE]),
                                op=OP.mult)
        nc.vector.tensor_scalar_min(ms_b[:], ms_b[:], 1.0)
        nc.vector.tensor_scalar_max(ms_b[:], ms_b[:], -1.0)
        m_b = smallp.tile([128, NTILE], f32, tag="mb")
        nc.vector.tensor_scalar(out=m_b[:], in0=ms_b[:], scalar1=MARGIN,
                                scalar2=MARGIN, op0=OP.mult, op1=OP.add)
        u_b = smallp.tile([128, NTILE], f32, tag="ub")
        nc.vector.tensor_tensor(out=u_b[:], in0=m_b[:], in1=m_b[:], op=OP.mult)
        sin_b = smallp.tile([128, NTILE], f32, tag="sinb")
        nc.vector.tensor_scalar(out=sin_b[:], in0=u_b[:], scalar1=1.0 / 120,
                                scalar2=-1.0 / 6, op0=OP.mult, op1=OP.add)
        nc.vector.tensor_tensor(out=sin_b[:], in0=sin_b[:], in1=u_b[:], op=OP.mult)
        nc.vector.tensor_scalar_add(sin_b[:], sin_b[:], 1.0)
        nc.vector.tensor_tensor(out=sin_b[:], in0=sin_b[:], in1=m_b[:], op=OP.mult)
        cos_b = smallp.tile([128, NTILE], f32, tag="cosb")
        nc.vector.tensor_scalar(out=cos_b[:], in0=u_b[:], scalar1=-1.0 / 720,
                                scalar2=1.0 / 24, op0=OP.mult, op1=OP.add)
        nc.vector.tensor_tensor(out=cos_b[:], in0=cos_b[:], in1=u_b[:], op=OP.mult)
        nc.vector.tensor_scalar_add(cos_b[:], cos_b[:], -0.5)
        nc.vector.tensor_tensor(out=cos_b[:], in0=cos_b[:], in1=u_b[:], op=OP.mult)
        nc.vector.tensor_scalar_add(cos_b[:], cos_b[:], 1.0)

        # ---- label path: cos_t from normalized gather rows (unit norm) ----
        dots_b = smallp.tile([128, NTILE], f32, tag="dots")
        for j in range(NTILE):
            scr2 = scrp.tile([128, D], f32, tag="sq")
            nc.vector.scalar_tensor_tensor(
                out=scr2[:], in0=emb_t[j][:], scalar=1.0, in1=wlab_t[j][:],
                op0=OP.mult, op1=OP.mult, accum_out=dots_b[:, j:j + 1])
        cost_b = smallp.tile([128, NTILE], f32, tag="cost")
        nc.vector.tensor_scalar_min(cost_b[:], dots_b[:], 1.0)
        nc.vector.tensor_scalar_max(cost_b[:], cost_b[:], -1.0)
        q_b = smallp.tile([128, NTILE], f32, tag="qb")
        nc.vector.tensor_tensor(out=q_b[:], in0=cost_b[:], in1=cost_b[:],
                                op=OP.mult)
        # rt = sqrt(1 - cos^2) = q*rsqrt(q) with q clamped away from 0
        nc.vector.tensor_scalar(out=q_b[:], in0=q_b[:], scalar1=-1.0,
                                scalar2=1.0, op0=OP.mult, op1=OP.add)
        nc.vector.tensor_scalar_max(q_b[:], q_b[:], 1e-12)
        rt_b = smallp.tile([128, NTILE], f32, tag="rtb")
        tmp3_b = scrp.tile([128, NTILE], f32, tag="tmpb")
        rsqrt(q_b[:], rt_b[:], tmp3_b[:])
        nc.vector.tensor_tensor(out=rt_b[:], in0=rt_b[:], in1=q_b[:],
                                op=OP.mult)
        costm_b = smallp.tile([128, NTILE], f32, tag="costm")
        nc.vector.tensor_tensor(out=costm_b[:], in0=cost_b[:], in1=cos_b[:],
                                op=OP.mult)
        nc.vector.tensor_tensor(out=rt_b[:], in0=rt_b[:], in1=sin_b[:],
                                op=OP.mult)
        nc.vector.tensor_tensor(out=costm_b[:], in0=costm_b[:], in1=rt_b[:],
                                op=OP.subtract)
        et_b = smallp.tile([128, NTILE], f32, tag="etb")
        nc.scalar.activation(et_b[:], cost_b[:], AF.Exp, bias=-30.0, scale=SCALEC)
        em_b = smallp.tile([128, NTILE], f32, tag="emb2")
        nc.scalar.activation(em_b[:], costm_b[:], AF.Exp, bias=-30.0, scale=SCALEC)
        corr_b = smallp.tile([128, NTILE], f32, tag="corrb")
        nc.vector.tensor_tensor(out=corr_b[:], in0=em_b[:], in1=et_b[:],
                                op=OP.subtract)
        nc.vector.tensor_tensor(out=corr_b[:], in0=corr_b[:], in1=valid_sb[:],
                                op=OP.mult)
        lab_b = smallp.tile([128, NTILE], f32, tag="labb")
        nc.scalar.mul(lab_b[:], costm_b[:], SCALEC)
        nc.vector.tensor_tensor(out=lab_b[:], in0=lab_b[:], in1=valid_sb[:],
                                op=OP.mult)

        # ---- collective 1: label logits only (hides under sweep; also
        # absorbs the cross-core start-skew rendezvous) ----
        cc1_in = dramp.tile([128, NTILE], f32, tag="cc1in")
        cc1_out = dramp.tile([128, NTILE], f32, tag="cc1out")
        nc.sync.dma_start(out=cc1_in[:], in_=lab_b[:])
        nc.gpsimd.collective_compute(
            "AllReduce", mybir.AluOpType.add,
            replica_groups=[list(range(NCORES))],
            ins=[cc1_in.opt()], outs=[cc1_out.opt()])
        cc1_res = smallp.tile([128, NTILE], f32, tag="cc1res")
        nc.sync.dma_start(out=cc1_res[:], in_=cc1_out[:])

        # ---- main sweep: g outer (DMA streaming order), j inner.
        # Most groups exp+reduce on ACT (accum_out); the j==3 column is
        # offloaded to DVE via Schraudolph fast-exp to balance engines.
        sums = constp.tile([128, NTILE * NGRP], f32, tag="sums")
        for g in range(NGRP):
            c0 = g * GRP
            w = GRP if g < NGRP_FULL else GRP_LAST
            nsub = (w + SUB - 1) // SUB
            for j in range(NTILE):
                ps = pmain.tile([128, GRP], f32, tag="ps")
                for kk in range(2):
                    for s in range(nsub):
                        ws = min(SUB, w - s * SUB)
                        nc.tensor.matmul(
                            out=ps[:, s * SUB:s * SUB + ws],
                            lhsT=embT8[kk][:, :, j * 128:(j + 1) * 128],
                            rhs=w8[:, kk, :, c0 + s * SUB:c0 + s * SUB + ws],
                            perf_mode=mybir.MatmulPerfMode.DoubleRow,
                            start=(kk == 0), stop=(kk == 1))
                col = sums[:, j * NGRP + g:j * NGRP + g + 1]
                if j == 3 and g >= 2:
                    ti = actp.tile([128, GRP], i32, tag="ti")
                    nc.vector.tensor_scalar(out=ti[:, 0:w], in0=ps[:, 0:w],
                                            scalar1=DVE_A, scalar2=DVE_B,
                                            op0=OP.mult, op1=OP.add)
                    nc.vector.reduce_sum(out=col, in_=ti[:, 0:w].bitcast(f32),
                                         axis=mybir.AxisListType.X)
                else:
                    ex = actp.tile([128, GRP], bf16, tag="ex")
                    nc.scalar.activation(ex[:, 0:w], ps[:, 0:w], AF.Exp,
                                         bias=-30.0, scale=S30,
                                         accum_out=col)

        # ---- per-sample totals (+ local label correction) + collective ----
        stot = smallp.tile([128, NTILE], f32, tag="stot")
        for j in range(NTILE):
            nc.vector.reduce_sum(out=stot[:, j:j + 1],
                                 in_=sums[:, j * NGRP:(j + 1) * NGRP],
                                 axis=mybir.AxisListType.X)
        nc.vector.tensor_tensor(out=stot[:], in0=stot[:], in1=corr_b[:],
                                op=OP.add)
        cc2_in = dramp.tile([128, NTILE], f32, tag="cc2in")
        cc2_out = dramp.tile([128, NTILE], f32, tag="cc2out")
        nc.sync.dma_start(out=cc2_in[:], in_=stot[:])
        nc.gpsimd.collective_compute(
            "AllReduce", mybir.AluOpType.add,
            replica_groups=[list(range(NCORES))],
            ins=[cc2_in.opt()], outs=[cc2_out.opt()])
        cc2_res = smallp.tile([128, NTILE], f32, tag="cc2res")
        nc.sync.dma_start(out=cc2_res[:], in_=cc2_out[:])

        lse_b = smallp.tile([128, NTILE], f32, tag="lseb")
        nc.scalar.activation(lse_b[:], cc2_res[:], AF.Ln, scale=EXP30)
        nc.vector.tensor_tensor(out=lse_b[:], in0=lse_b[:],
                                in1=cc1_res[:], op=OP.subtract)
        part = smallp.tile([128, 1], f32, tag="part")
        nc.vector.reduce_sum(out=part[:], in_=lse_b[:], axis=mybir.AxisListType.X)
        ps_l = pmisc.tile([1, 1], f32, tag="misc")
        nc.tensor.matmul(out=ps_l[:], lhsT=ones_f[:], rhs=part[:],
                         start=True, stop=True)
        loss_sb = smallp.tile([1, 1], f32, tag="loss")
        nc.scalar.mul(loss_sb[:], ps_l[:], 1.0 / N)
        nc.sync.dma_start(out=out_d[:, :], in_=loss_sb[:])

    nc.finalize()
    return nc


def _host_prep(embeddings, labels, weight):
    import ml_dtypes
    emb = np.ascontiguousarray(embeddings, dtype=np.float32)
    w = np.ascontiguousarray(weight, dtype=np.float32)
    lab = np.asarray(labels).astype(np.int64)
    # normalize rows once for the full weight matrix
    wn = np.sqrt((w * w).sum(axis=1, keepdims=True))
    wu = w / wn
    # k-major fp8 layout for the whole matrix: [128(p), 2(kk), 2(o), C]
    # with k = kk*256 + o*128 + p
    wt8_full = np.ascontiguousarray(
        (wu.T * np.float32(FP8S)).reshape(2, 2, 128, C).transpose(2, 0, 1, 3)
    ).astype(ml_dtypes.float8_e4m3)
    ident_bf = np.eye(128, dtype=ml_dtypes.bfloat16)
    ones_f = np.ones((128, 1), dtype=np.float32)
    ones_row = np.ones((1, 128), dtype=np.float32)
    in_maps = []
    for core in range(NCORES):
        lab_loc = lab - core * CLOC
        valid = ((lab_loc >= 0) & (lab_loc < CLOC)).astype(np.float32)
        idx = np.clip(lab_loc, 0, CLOC - 1).astype(np.int32)
        in_maps.append({
            "wt8": np.ascontiguousarray(
                wt8_full[:, :, :, core * CLOC:(core + 1) * CLOC]),
            "wrows": np.ascontiguousarray(wu[core * CLOC:(core + 1) * CLOC]),
            "emb": emb,
            "labidx": np.ascontiguousarray(idx.reshape(NTILE, 128).T),
            "valid": np.ascontiguousarray(valid.reshape(NTILE, 128).T),
            "identbf": ident_bf,
            "onesf": ones_f,
            "onesrow": ones_row,
        })
    return in_maps


def run(embeddings, labels, weight, trace=False):
    from concourse import bass_utils
    if "nc" not in _cache:
        _cache["nc"] = _build()
    in_maps = _host_prep(embeddings, labels, weight)
    res = bass_utils.run_bass_kernel_spmd(
        _cache["nc"], in_maps, core_ids=list(range(NCORES)), trace=trace)
    out = np.asarray(res.results[0]["out"], dtype=np.float32).reshape(())
    return out, res


def kernel(embeddings, labels, weight):
    out, _ = run(embeddings, labels, weight, trace=False)
    return out


# revision 8
# speedup vs baseline: 1.7721x; 1.0518x over previous
"""AdaFace loss on 8 TRN2 NeuronCores — class-parallel margin softmax.

Sharding: class dim split 12500/core. Host pre-normalizes weight rows and
casts to fp8 in k-major DoubleRow layout [128, 2(kk), 2(o), 12500]; the
device streams W from HBM (6.4MB/core) into one big SBUF tile via chunked
DMAs. Matmul keeps the (transposed, normalized, fp8) embeddings stationary
and streams W: psum[128 batch, <=1536 classes] accumulates K=512 in 2
DoubleRow matmuls per 512-class sub-chunk. ACT then applies
exp(30*cos - 30) with accum_out producing per-sample sumexp partials
directly. Label-margin terms come from an f32 row gather and are combined
via an early AllReduce that hides under the main sweep; a warmup AllReduce
at t~0 absorbs core-start skew; the final sumexp AllReduce is the only
exposed collective.
"""
import math
import numpy as np

NCORES = 8
C, D, N = 100000, 512, 512
CLOC = C // NCORES            # 12500
SUB = 512                     # classes per matmul / psum bank
GRP = 3 * SUB                 # classes per psum tile / ACT exp (3 banks)
NGRP_FULL = CLOC // GRP       # 8 full groups
GRP_LAST = CLOC - NGRP_FULL * GRP   # 212
NGRP = NGRP_FULL + 1          # 9
NTILE = N // 128              # 4 batch tiles
SCALEC = 30.0
MARGIN = 0.4
HCONST = 0.333
FP8S = 16.0                   # fp8 scaling for both operands
S30 = SCALEC / (FP8S * FP8S)  # activation scale: psum = 256*cos
EXP30 = float(np.exp(np.float32(30.0)))
# Schraudolph fast-exp: exp(y) ~= bitcast_f32(int(y*FEA + FEB)); for the
# DVE-offloaded groups y = S30*psum - 30, so i = psum*(FEA*S30) + (FEB - 30*FEA)
FEA = 12102203.161561485      # 2^23/ln(2)
FEB = 1064866805.0
DVE_A = FEA * S30
DVE_B = FEB - 30.0 * FEA

_cache = {}


def _build():
    import concourse.bass as bass
    import concourse.bacc as bacc
    import concourse.mybir as mybir
    import concourse.tile as tile
    from contextlib import ExitStack

    f32 = mybir.dt.float32
    bf16 = mybir.dt.bfloat16
    fp8 = mybir.dt.float8e4
    i32 = mybir.dt.int32
    AF = mybir.ActivationFunctionType
    OP = mybir.AluOpType

    nc = bacc.Bacc("TRN2", target_bir_lowering=False, debug=False,
                   num_devices=NCORES)
    _c30 = nc.alloc_sbuf_tensor("const-f32-neg30", [128, 1], f32)
    nc.gpsimd.memset(_c30.ap(), -30.0)
    nc.const_aps.aps[(f32, -30.0)] = _c30.ap()
    nc.all_engine_barrier()

    wt8_d = nc.dram_tensor("wt8", [128, 2, 2, CLOC], fp8, kind="ExternalInput")
    wrows_d = nc.dram_tensor("wrows", [CLOC, D], f32, kind="ExternalInput")
    emb_d = nc.dram_tensor("emb", [N, D], f32, kind="ExternalInput")
    labidx_d = nc.dram_tensor("labidx", [128, NTILE], i32, kind="ExternalInput")
    valid_d = nc.dram_tensor("valid", [128, NTILE], f32, kind="ExternalInput")
    identbf_d = nc.dram_tensor("identbf", [128, 128], bf16, kind="ExternalInput")
    onesf_d = nc.dram_tensor("onesf", [128, 1], f32, kind="ExternalInput")
    onesrow_d = nc.dram_tensor("onesrow", [1, 128], f32, kind="ExternalInput")
    out_d = nc.dram_tensor("out", [1, 1], f32, kind="ExternalOutput")

    with tile.TileContext(nc) as tc, ExitStack() as ctx:
        constp = ctx.enter_context(tc.tile_pool(name="const", bufs=1))
        scrp = ctx.enter_context(tc.tile_pool(name="scratch", bufs=2))
        actp = ctx.enter_context(tc.tile_pool(name="actout", bufs=2))
        smallp = ctx.enter_context(tc.tile_pool(name="small", bufs=2))
        pmain = ctx.enter_context(tc.tile_pool(name="pmain", bufs=2, space="PSUM"))
        pmisc = ctx.enter_context(tc.tile_pool(name="pmisc", bufs=2, space="PSUM"))
        dramp = ctx.enter_context(tc.tile_pool(name="dram", bufs=1, space="DRAM"))

        # ---- everything on the sync queue (it cold-starts fastest):
        # embeddings + small consts first (they gate the prep chain), then
        # the 9 group-aligned W-shard chunks stream behind them.
        emb_t = []
        for j in range(NTILE):
            t = constp.tile([128, D], f32, tag=f"emb{j}")
            nc.sync.dma_start(out=t[:], in_=emb_d[j * 128:(j + 1) * 128, :])
            emb_t.append(t)
        ident_bf = constp.tile([128, 128], bf16, tag="identbf")
        nc.sync.dma_start(out=ident_bf[:], in_=identbf_d[:, :])
        ones_f = constp.tile([128, 1], f32, tag="onesf")
        nc.sync.dma_start(out=ones_f[:], in_=onesf_d[:, :])
        ones_row = constp.tile([1, 128], f32, tag="onesrow")
        nc.sync.dma_start(out=ones_row[:], in_=onesrow_d[:, :])
        idx_sb = constp.tile([128, NTILE], i32, tag="idx")
        nc.sync.dma_start(out=idx_sb[:], in_=labidx_d[:, :])
        valid_sb = constp.tile([128, NTILE], f32, tag="valid")
        nc.sync.dma_start(out=valid_sb[:], in_=valid_d[:, :])

        w8 = constp.tile([128, 2, 2, CLOC], fp8, tag="w8")
        for g in range(NGRP):
            c0 = g * GRP
            w = GRP if g < NGRP_FULL else GRP_LAST
            nc.sync.dma_start(out=w8[:, :, :, c0:c0 + w],
                              in_=wt8_d[:, :, :, c0:c0 + w])

        def rsqrt(x_ap, y_ap, t_ap, iters=2):
            """y = 1/sqrt(x) via bitcast seed + Newton (x > 0)."""
            xi = x_ap.bitcast(i32)
            yi = y_ap.bitcast(i32)
            nc.vector.tensor_scalar(out=yi, in0=xi, scalar1=1, scalar2=None,
                                    op0=OP.arith_shift_right)
            nc.vector.tensor_scalar(out=yi, in0=yi, scalar1=-1,
                                    scalar2=0x5f3759df, op0=OP.mult, op1=OP.add)
            for _ in range(iters):
                nc.vector.tensor_tensor(out=t_ap, in0=x_ap, in1=y_ap, op=OP.mult)
                nc.vector.tensor_tensor(out=t_ap, in0=t_ap, in1=y_ap, op=OP.mult)
                nc.vector.tensor_scalar(out=t_ap, in0=t_ap, scalar1=-0.5,
                                        scalar2=1.5, op0=OP.mult, op1=OP.add)
                nc.vector.tensor_tensor(out=y_ap, in0=y_ap, in1=t_ap, op=OP.mult)

        # ---- label-row gather (early: overlaps weight DMA) ----
        wlab_t = []
        for j in range(NTILE):
            wl = constp.tile([128, D], f32, tag=f"wlab{j}")
            nc.gpsimd.indirect_dma_start(
                out=wl[:], out_offset=None, in_=wrows_d[:, :],
                in_offset=bass.IndirectOffsetOnAxis(ap=idx_sb[:, j:j + 1],
                                                    axis=0))
            wlab_t.append(wl)

        # ---- embedding prep ----
        norms2_b = constp.tile([128, NTILE], f32, tag="norms2")
        for j in range(NTILE):
            scr = scrp.tile([128, D], f32, tag="sq")
            nc.vector.scalar_tensor_tensor(
                out=scr[:], in0=emb_t[j][:], scalar=1.0, in1=emb_t[j][:],
                op0=OP.mult, op1=OP.mult, accum_out=norms2_b[:, j:j + 1])
        invn_b = constp.tile([128, NTILE], f32, tag="invn")
        tmp_b = scrp.tile([128, NTILE], f32, tag="tmpb")
        rsqrt(norms2_b[:], invn_b[:], tmp_b[:])                # 1/||e||
        norms_b = constp.tile([128, NTILE], f32, tag="norms")
        nc.vector.tensor_tensor(out=norms_b[:], in0=norms2_b[:], in1=invn_b[:],
                                op=OP.mult)                    # ||e||

        # normalize, scale x16 to bf16, transpose into fp8 embT8[kk][p,o,n]
        embT8 = [constp.tile([128, 2, N], fp8, name=f"embT8_{kk}",
                             tag=f"embT8_{kk}") for kk in range(2)]
        for j in range(NTILE):
            nc.vector.tensor_scalar_mul(emb_t[j][:], emb_t[j][:],
                                        invn_b[:, j:j + 1])
            e16 = scrp.tile([128, D], bf16, tag="e16")
            nc.vector.tensor_scalar(out=e16[:], in0=emb_t[j][:],
                                    scalar1=FP8S, scalar2=None, op0=OP.mult)
            for k4 in range(4):
                pst = pmisc.tile([128, 128], bf16, tag="misc")
                nc.tensor.transpose(out=pst[:],
                                    in_=e16[:, k4 * 128:(k4 + 1) * 128],
                                    identity=ident_bf[:])
                nc.vector.tensor_copy(
                    out=embT8[k4 // 2][:, k4 % 2, j * 128:(j + 1) * 128],
                    in_=pst[:])

        # ---- batch stats: mean/std of ||e|| ----
        stat_in = smallp.tile([128, 2 * NTILE], f32, tag="statin")
        nc.vector.tensor_copy(out=stat_in[:, 0:NTILE], in_=norms_b[:])
        nc.vector.tensor_copy(out=stat_in[:, NTILE:2 * NTILE], in_=norms2_b[:])
        ps_stat = pmisc.tile([1, 2 * NTILE], f32, tag="misc")
        nc.tensor.matmul(out=ps_stat[:], lhsT=ones_f[:], rhs=stat_in[:],
                         start=True, stop=True)
        sums2 = smallp.tile([1, 2], f32, tag="sums2")
        nc.vector.reduce_sum(out=sums2[:, 0:1], in_=ps_stat[:, 0:NTILE],
                             axis=mybir.AxisListType.X)
        nc.vector.reduce_sum(out=sums2[:, 1:2], in_=ps_stat[:, NTILE:2 * NTILE],
                             axis=mybir.AxisListType.X)
        # mean = S1/N ; var = (S2 - S1^2/N)/(N-1); std = sqrt(var)
        scal = smallp.tile([1, 4], f32, tag="scal")
        nc.scalar.mul(scal[:, 0:1], sums2[:, 0:1], 1.0 / N)           # mean
        nc.vector.tensor_tensor(out=scal[:, 1:2], in0=sums2[:, 0:1],
                                in1=sums2[:, 0:1], op=OP.mult)        # S1^2
        nc.scalar.mul(scal[:, 1:2], scal[:, 1:2], 1.0 / N)            # S1^2/N
        nc.vector.tensor_tensor(out=scal[:, 1:2], in0=sums2[:, 1:2],
                                in1=scal[:, 1:2], op=OP.subtract)
        nc.scalar.mul(scal[:, 1:2], scal[:, 1:2], 1.0 / (N - 1))      # var
        sct = smallp.tile([1, 2], f32, tag="sct")
        rsqrt(scal[:, 1:2], scal[:, 2:3], sct[:, 0:1])                # 1/std
        nc.vector.tensor_tensor(out=scal[:, 2:3], in0=scal[:, 1:2],
                                in1=scal[:, 2:3], op=OP.mult)         # std
        nc.vector.tensor_scalar_add(scal[:, 2:3], scal[:, 2:3], HCONST)  # std+H
        nc.vector.reciprocal(out=scal[:, 3:4], in_=scal[:, 2:3])      # 1/(std+H)
        # broadcast mean, 1/(std+H) to 128 partitions via K=1 outer product
        bvals = smallp.tile([1, 2], f32, tag="bvals")
        nc.vector.tensor_copy(out=bvals[:, 0:1], in_=scal[:, 0:1])
        nc.vector.tensor_copy(out=bvals[:, 1:2], in_=scal[:, 3:4])
        ps_bc = pmisc.tile([128, 2], f32, tag="misc")
        nc.tensor.matmul(out=ps_bc[:], lhsT=ones_row[:], rhs=bvals[:],
                         start=True, stop=True)
        bc = smallp.tile([128, 2], f32, tag="bc")
        nc.vector.tensor_copy(out=bc[:], in_=ps_bc[:])

        # margin scaler -> m, sin(m), cos(m) (poly, avoids Sin table set)
        ms_b = smallp.tile([128, NTILE], f32, tag="msb")
        nc.vector.tensor_tensor(out=ms_b[:], in0=norms_b[:],
                                in1=bc[:, 0:1].to_broadcast([128, NTILE]),
                                op=OP.subtract)
        nc.vector.tensor_tensor(out=ms_b[:], in0=ms_b[:],
                                in1=bc[:, 1:2].to_broadcast([128, NTILE]),
                                op=OP.mult)
        nc.vector.tensor_scalar_min(ms_b[:], ms_b[:], 1.0)
        nc.vector.tensor_scalar_max(ms_b[:], ms_b[:], -1.0)
        m_b = smallp.tile([128, NTILE], f32, tag="mb")
        nc.vector.tensor_scalar(out=m_b[:], in0=ms_b[:], scalar1=MARGIN,
                                scalar2=MARGIN, op0=OP.mult, op1=OP.add)
        u_b = smallp.tile([128, NTILE], f32, tag="ub")
        nc.vector.tensor_tensor(out=u_b[:], in0=m_b[:], in1=m_b[:], op=OP.mult)
        sin_b = smallp.tile([128, NTILE], f32, tag="sinb")
        nc.vector.tensor_scalar(out=sin_b[:], in0=u_b[:], scalar1=1.0 / 120,
                                scalar2=-1.0 / 6, op0=OP.mult, op1=OP.add)
        nc.vector.tensor_tensor(out=sin_b[:], in0=sin_b[:], in1=u_b[:], op=OP.mult)
        nc.vector.tensor_scalar_add(sin_b[:], sin_b[:], 1.0)
        nc.vector.tensor_tensor(out=sin_b[:], in0=sin_b[:], in1=m_b[:], op=OP.mult)
        cos_b = smallp.tile([128, NTILE], f32, tag="cosb")
        nc.vector.tensor_scalar(out=cos_b[:], in0=u_b[:], scalar1=-1.0 / 720,
                                scalar2=1.0 / 24, op0=OP.mult, op1=OP.add)
        nc.vector.tensor_tensor(out=cos_b[:], in0=cos_b[:], in1=u_b[:], op=OP.mult)
        nc.vector.tensor_scalar_add(cos_b[:], cos_b[:], -0.5)
        nc.vector.tensor_tensor(out=cos_b[:], in0=cos_b[:], in1=u_b[:], op=OP.mult)
        nc.vector.tensor_scalar_add(cos_b[:], cos_b[:], 1.0)

        # ---- label path: cos_t from normalized gather rows (unit norm) ----
        dots_b = smallp.tile([128, NTILE], f32, tag="dots")
        for j in range(NTILE):
            scr2 = scrp.tile([128, D], f32, tag="sq")
            nc.vector.scalar_tensor_tensor(
                out=scr2[:], in0=emb_t[j][:], scalar=1.0, in1=wlab_t[j][:],
                op0=OP.mult, op1=OP.mult, accum_out=dots_b[:, j:j + 1])
        cost_b = smallp.tile([128, NTILE], f32, tag="cost")
        nc.vector.tensor_scalar_min(cost_b[:], dots_b[:], 1.0)
        nc.vector.tensor_scalar_max(cost_b[:], cost_b[:], -1.0)
        q_b = smallp.tile([128, NTILE], f32, tag="qb")
        nc.vector.tensor_tensor(out=q_b[:], in0=cost_b[:], in1=cost_b[:],
                                op=OP.mult)
        # rt = sqrt(1 - cos^2) = q*rsqrt(q) with q clamped away from 0
        nc.vector.tensor_scalar(out=q_b[:], in0=q_b[:], scalar1=-1.0,
                                scalar2=1.0, op0=OP.mult, op1=OP.add)
        nc.vector.tensor_scalar_max(q_b[:], q_b[:], 1e-12)
        rt_b = smallp.tile([128, NTILE], f32, tag="rtb")
        tmp3_b = scrp.tile([128, NTILE], f32, tag="tmpb")
        rsqrt(q_b[:], rt_b[:], tmp3_b[:])
        nc.vector.tensor_tensor(out=rt_b[:], in0=rt_b[:], in1=q_b[:],
                                op=OP.mult)
        costm_b = smallp.tile([128, NTILE], f32, tag="costm")
        nc.vector.tensor_tensor(out=costm_b[:], in0=cost_b[:], in1=cos_b[:],
                                op=OP.mult)
        nc.vector.tensor_tensor(out=rt_b[:], in0=rt_b[:], in1=sin_b[:],
                                op=OP.mult)
        nc.vector.tensor_tensor(out=costm_b[:], in0=costm_b[:], in1=rt_b[:],
                                op=OP.subtract)
        et_b = smallp.tile([128, NTILE], f32, tag="etb")
        nc.scalar.activation(et_b[:], cost_b[:], AF.Exp, bias=-30.0, scale=SCALEC)
        em_b = smallp.tile([128, NTILE], f32, tag="emb2")
        nc.scalar.activation(em_b[:], costm_b[:], AF.Exp, bias=-30.0, scale=SCALEC)
        corr_b = smallp.tile([128, NTILE], f32, tag="corrb")
        nc.vector.tensor_tensor(out=corr_b[:], in0=em_b[:], in1=et_b[:],
                                op=OP.subtract)
        nc.vector.tensor_tensor(out=corr_b[:], in0=corr_b[:], in1=valid_sb[:],
                                op=OP.mult)
        lab_b = smallp.tile([128, NTILE], f32, tag="labb")
        nc.scalar.mul(lab_b[:], costm_b[:], SCALEC)
        nc.vector.tensor_tensor(out=lab_b[:], in0=lab_b[:], in1=valid_sb[:],
                                op=OP.mult)

        # ---- collective 1: label logits only (hides under sweep; also
        # absorbs the cross-core start-skew rendezvous) ----
        cc1_in = dramp.tile([128, NTILE], f32, tag="cc1in")
        cc1_out = dramp.tile([128, NTILE], f32, tag="cc1out")
        nc.sync.dma_start(out=cc1_in[:], in_=lab_b[:])
        nc.gpsimd.collective_compute(
            "AllReduce", mybir.AluOpType.add,
            replica_groups=[list(range(NCORES))],
            ins=[cc1_in.opt()], outs=[cc1_out.opt()])
        cc1_res = smallp.tile([128, NTILE], f32, tag="cc1res")
        nc.sync.dma_start(out=cc1_res[:], in_=cc1_out[:])

        # ---- main sweep: g outer (DMA streaming order), j inner.
        # Each psum group is consumed by BOTH engines in parallel: ACT does
        # exp(+accum_out) on the first ASPLIT cols, DVE does Schraudolph
        # fast-exp + reduce on the rest — so the consumer keeps pace with PE.
        ASPLIT = 896
        sums = constp.tile([128, NTILE * NGRP * 2], f32, tag="sums")
        nc.vector.memset(sums[:], 0.0)
        for g in range(NGRP):
            c0 = g * GRP
            w = GRP if g < NGRP_FULL else GRP_LAST
            nsub = (w + SUB - 1) // SUB
            for j in range(NTILE):
                ps = pmain.tile([128, GRP], f32, tag="ps")
                for kk in range(2):
                    for s in range(nsub):
                        ws = min(SUB, w - s * SUB)
                        nc.tensor.matmul(
                            out=ps[:, s * SUB:s * SUB + ws],
                            lhsT=embT8[kk][:, :, j * 128:(j + 1) * 128],
                            rhs=w8[:, kk, :, c0 + s * SUB:c0 + s * SUB + ws],
                            perf_mode=mybir.MatmulPerfMode.DoubleRow,
                            start=(kk == 0), stop=(kk == 1))
                base = (j * NGRP + g) * 2
                wa = min(ASPLIT, w)
                ex = actp.tile([128, ASPLIT], bf16, tag="ex")
                nc.scalar.activation(ex[:, 0:wa], ps[:, 0:wa], AF.Exp,
                                     bias=-30.0, scale=S30,
                                     accum_out=sums[:, base:base + 1])
                if w > ASPLIT:
                    ti = actp.tile([128, GRP - ASPLIT], i32, tag="ti")
                    nc.vector.tensor_scalar(out=ti[:, 0:w - ASPLIT],
                                            in0=ps[:, ASPLIT:w],
                                            scalar1=DVE_A, scalar2=DVE_B,
                                            op0=OP.mult, op1=OP.add)
                    nc.vector.reduce_sum(
                        out=sums[:, base + 1:base + 2],
                        in_=ti[:, 0:w - ASPLIT].bitcast(f32),
                        axis=mybir.AxisListType.X)

        # ---- per-sample totals (+ local label correction) + collective ----
        stot = smallp.tile([128, NTILE], f32, tag="stot")
        for j in range(NTILE):
            nc.vector.reduce_sum(out=stot[:, j:j + 1],
                                 in_=sums[:, j * NGRP * 2:(j + 1) * NGRP * 2],
                                 axis=mybir.AxisListType.X)
        nc.vector.tensor_tensor(out=stot[:], in0=stot[:], in1=corr_b[:],
                                op=OP.add)
        cc2_in = dramp.tile([128, NTILE], f32, tag="cc2in")
        cc2_out = dramp.tile([128, NTILE], f32, tag="cc2out")
        nc.sync.dma_start(out=cc2_in[:], in_=stot[:])
        nc.gpsimd.collective_compute(
            "AllReduce", mybir.AluOpType.add,
            replica_groups=[list(range(NCORES))],
            ins=[cc2_in.opt()], outs=[cc2_out.opt()])
        cc2_res = smallp.tile([128, NTILE], f32, tag="cc2res")
        nc.sync.dma_start(out=cc2_res[:], in_=cc2_out[:])

        lse_b = smallp.tile([128, NTILE], f32, tag="lseb")
        nc.scalar.activation(lse_b[:], cc2_res[:], AF.Ln, scale=EXP30)
        nc.vector.tensor_tensor(out=lse_b[:], in0=lse_b[:],
                                in1=cc1_res[:], op=OP.subtract)
        part = smallp.tile([128, 1], f32, tag="part")
        nc.vector.reduce_sum(out=part[:], in_=lse_b[:], axis=mybir.AxisListType.X)
        ps_l = pmisc.tile([1, 1], f32, tag="misc")
        nc.tensor.matmul(out=ps_l[:], lhsT=ones_f[:], rhs=part[:],
                         start=True, stop=True)
        loss_sb = smallp.tile([1, 1], f32, tag="loss")
        nc.scalar.mul(loss_sb[:], ps_l[:], 1.0 / N)
        nc.sync.dma_start(out=out_d[:, :], in_=loss_sb[:])

    nc.finalize()
    return nc


def _host_prep(embeddings, labels, weight):
    import ml_dtypes
    emb = np.ascontiguousarray(embeddings, dtype=np.float32)
    w = np.ascontiguousarray(weight, dtype=np.float32)
    lab = np.asarray(labels).astype(np.int64)
    # normalize rows once for the full weight matrix
    wn = np.sqrt((w * w).sum(axis=1, keepdims=True))
    wu = w / wn
    # k-major fp8 layout for the whole matrix: [128(p), 2(kk), 2(o), C]
    # with k = kk*256 + o*128 + p
    wt8_full = np.ascontiguousarray(
        (wu.T * np.float32(FP8S)).reshape(2, 2, 128, C).transpose(2, 0, 1, 3)
    ).astype(ml_dtypes.float8_e4m3)
    ident_bf = np.eye(128, dtype=ml_dtypes.bfloat16)
    ones_f = np.ones((128, 1), dtype=np.float32)
    ones_row = np.ones((1, 128), dtype=np.float32)
    in_maps = []
    for core in range(NCORES):
        lab_loc = lab - core * CLOC
        valid = ((lab_loc >= 0) & (lab_loc < CLOC)).astype(np.float32)
        idx = np.clip(lab_loc, 0, CLOC - 1).astype(np.int32)
        in_maps.append({
            "wt8": np.ascontiguousarray(
                wt8_full[:, :, :, core * CLOC:(core + 1) * CLOC]),
            "wrows": np.ascontiguousarray(wu[core * CLOC:(core + 1) * CLOC]),
            "emb": emb,
            "labidx": np.ascontiguousarray(idx.reshape(NTILE, 128).T),
            "valid": np.ascontiguousarray(valid.reshape(NTILE, 128).T),
            "identbf": ident_bf,
            "onesf": ones_f,
            "onesrow": ones_row,
        })
    return in_maps


def run(embeddings, labels, weight, trace=False):
    from concourse import bass_utils
    if "nc" not in _cache:
        _cache["nc"] = _build()
    in_maps = _host_prep(embeddings, labels, weight)
    res = bass_utils.run_bass_kernel_spmd(
        _cache["nc"], in_maps, core_ids=list(range(NCORES)), trace=trace)
    out = np.asarray(res.results[0]["out"], dtype=np.float32).reshape(())
    return out, res


def kernel(embeddings, labels, weight):
    out, _ = run(embeddings, labels, weight, trace=False)
    return out
